# revision 20
# baseline (speedup 1.0000x reference)
"""Trainium2 Bass kernel for a dense transformer block (B=4, T=2048, C=1024,
H=16, FF=2048) with a random-permuted causal mask.

Strategy (8 NeuronCores, SPMD, collective-free):
  - 2 cores per batch; each core owns 1024 query rows = two global 512-row
    q-blocks, picked so causal work balances: half0 -> blocks {0,3},
    half1 -> blocks {1,2}.  Uniform program: block A runs 8 key-tile slots,
    block B 16 slots; per-core mask data zeroes the slots a core doesn't need.
  - Keys are processed in perm-sorted order (sigma = argsort(perm)), which
    turns the permuted mask into a standard causal mask -> block skipping.
  - The residual stream is kept feature-major (transposed) on chip so the
    layernorms fold into the matmuls:  LN gain g folds into weights host-side,
    the mean folds in as an extra K=1 contraction row, and 1/std is applied
    multiplicatively at PSUM eviction.  No on-chip transposes at all.
  - Each core recomputes K/V for its full batch from a host-permuted,
    host-transposed copy of x (no inter-core communication anywhere).
  - q/k/v/y live in DRAM between phases (SBUF is the scarce resource).
Output is returned feature-major per core and transposed on host.
"""

import os
import sys
from contextlib import ExitStack

import numpy as np

for _p in ("/opt/trn_rl_repo",):
    if os.path.isdir(_p) and _p not in sys.path:
        sys.path.insert(0, _p)

import ml_dtypes

import concourse.bass as bass
import concourse.mybir as mybir
import concourse.tile as tile
from concourse import bacc
from concourse.bass_utils import run_bass_kernel_spmd

BF16 = ml_dtypes.bfloat16
F32 = np.float32

B, T, C, H, D, FF = 4, 2048, 1024, 16, 64, 2048
EPS = 1e-5
NCORE = 8
CT = C // 128          # 8 contraction tiles over C
FT = FF // 128         # 16 tiles over FF
NPAIR = H // 2         # 8 head pairs
TOWN = 1024            # query tokens owned per core
SLOTS = (8, 16)        # key-tile slots for q-block A / q-block B
BLOCKS = {0: (0, 3), 1: (1, 2)}   # half -> (global q-block A, B)

LAST_RESULT = None     # BassKernelResults of the last run (for test harness)

f32 = mybir.dt.float32
bf = mybir.dt.bfloat16
AF = mybir.ActivationFunctionType
OP = mybir.AluOpType


# --------------------------------------------------------------------------
# program builder
# --------------------------------------------------------------------------

def _emit(tc, P, flags):
    nc = tc.nc
    es = ExitStack()

    with es:
        const = es.enter_context(tc.tile_pool(name="const", bufs=1))
        dram = es.enter_context(tc.tile_pool(name="dram", bufs=1, space="DRAM"))
        ps_mm = es.enter_context(tc.tile_pool(name="ps_mm", bufs=2, space="PSUM"))
        statw = es.enter_context(tc.tile_pool(name="statw", bufs=2))
        statr = es.enter_context(tc.tile_pool(name="statr", bufs=1))

        onescol = const.tile([128, 1], bf)
        nc.vector.memset(onescol, 1.0)
        epscol = const.tile([128, 1], f32)
        nc.vector.memset(epscol, EPS)
        onesrow_bf = const.tile([1, 128], bf)
        nc.vector.memset(onesrow_bf, 1.0)
        onesrow_32 = const.tile([1, 128], f32)
        nc.vector.memset(onesrow_32, 1.0)

        def bcast_rows(row_ap, out_tile, ones_row):
            """out_tile[p, :] = row_ap[0, :] for all p, via PE outer product."""
            np_ = out_tile.shape[0]
            psb = ps_mm.tile([128, 512], f32, tag="mm")
            nc.tensor.matmul(psb[0:np_, :], ones_row[0:1, 0:np_], row_ap,
                             start=True, stop=True, tile_position=(0, 0))
            nc.vector.tensor_copy(out=out_tile, in_=psb[0:np_, :])

        def dma_in3(dst, dram_ap):
            """Split a [128, n, W] load into per-plane DMAs so no consumer
            needs more sync waits than one instruction can encode."""
            n = dst.shape[1]
            for a in range(n):
                nc.sync.dma_start(out=dst[:, a, :], in_=dram_ap[:, a, :])

        # per-token scalar rows.  Rows that feed matmuls (as K=1 contraction
        # rows) must sit at base partition 0 -> own [1, N] tiles.  Rows only
        # read by DMA/DVE are packed into shared [128, T] tiles.
        muq_row_t = const.tile([1, TOWN], bf)
        murs2_row_t = const.tile([1, TOWN], bf)
        rs2bf_row_t = const.tile([1, TOWN], bf)
        mu2_row_t = const.tile([1, TOWN], bf)
        rsq_row_t = const.tile([1, TOWN], f32)
        rs2_row_t = const.tile([1, TOWN], f32)
        mursf_row_t = const.tile([1, TOWN], f32)
        if flags["b1"]:
            stdq_row_t = const.tile([1, TOWN], bf)

        # DRAM bounce tensors (qT/yT only; kT/v stay SBUF-resident)
        qT_d = dram.tile([128, NPAIR, TOWN], bf)
        yT_d = dram.tile([128, NPAIR, TOWN], bf)

        def ln_stats(xT, ntok, murow, rsrow, rs_cols, stdrow, row_off=0):
            """Feature-major LN stats over [row_off, row_off+ntok).  xT is the
            FULL [128, CT, *] bf16 tensor; murow (bf16) / rsrow (f32) /
            stdrow (bf16) are full-width [1, *] row APs; rs_cols optionally
            gets the [128, ntok//128] f32 column form (token t ->
            [t % 128, t // 128]) via a DRAM-bounce reshape."""
            nch = ntok // 512
            for ci in range(nch):
                qs = slice(row_off + 512 * ci, row_off + 512 * ci + 512)
                ps = ps_mm.tile([128, 512], f32, tag="mm")
                for ct in range(CT):
                    xs = xT[:, ct, qs]
                    sq = statw.tile([128, 512], bf, tag="st_sq")
                    nc.vector.tensor_tensor(out=sq[:], in0=xs, in1=xs, op=OP.mult)
                    nc.tensor.matmul(ps[0:1, :], onescol[:], xs,
                                     start=(ct == 0), stop=(ct == CT - 1),
                                     tile_position=(0, 0))
                    nc.tensor.matmul(ps[32:33, :], onescol[:], sq[:],
                                     start=(ct == 0), stop=(ct == CT - 1),
                                     tile_position=(0, 32))
                # row-form stats for this 512-token chunk
                muf = statr.tile([1, 512], f32, tag="st_muf")
                nc.scalar.mul(muf[:], ps[0:1, :], 1.0 / C)
                musq = statr.tile([1, 512], f32, tag="st_musq")
                nc.vector.tensor_tensor(out=musq[:], in0=muf[:], in1=muf[:],
                                        op=OP.mult)
                var = statr.tile([1, 512], f32, tag="st_var")
                nc.vector.scalar_tensor_tensor(out=var[:], in0=ps[32:33, :],
                                               scalar=1.0 / C, in1=musq[:],
                                               op0=OP.mult, op1=OP.subtract)
                std = statr.tile([1, 512], f32, tag="st_std")
                nc.scalar.activation(std[:], var[:], AF.Sqrt,
                                     bias=epscol[0:1, :])
                scr = statr.tile([1, 512], f32, tag="st_scr")
                nc.vector.reciprocal_approx_accurate(out=rsrow[:, qs],
                                                     in_=std[:], scratch=scr[:])
                nc.vector.tensor_copy(out=murow[:, qs], in_=muf[:])
                if stdrow is not None:
                    nc.vector.tensor_copy(out=stdrow[:, qs], in_=std[:])
            if rs_cols is not None:
                scratch_d = dram.tile([ntok], f32, tag="st_dram")
                nc.sync.dma_start(out=scratch_d[:], in_=rsrow[:, 0:ntok])
                nc.sync.dma_start(
                    out=rs_cols,
                    in_=scratch_d.rearrange("(j p) -> p j", p=128))

        muq_row = muq_row_t[0:1, 0:TOWN]
        murs2_row = murs2_row_t[0:1, 0:TOWN]
        rs2bf_row = rs2bf_row_t[0:1, 0:TOWN]
        stdq_row = stdq_row_t[0:1, 0:TOWN] if flags["b1"] else None
        rsq_row = rsq_row_t[0:1, 0:TOWN]
        rs2_row = rs2_row_t[0:1, 0:TOWN]
        mursf_row = mursf_row_t[0:1, 0:TOWN]

        # ------------------------------------------------------------------
        # Phase 1: stats + K + V from x_perm^T, then Q from x^T.
        # The Q-phase inputs are prefetched up front (DMA is otherwise idle
        # during the K/V matmuls).
        # ------------------------------------------------------------------
        wlate = es.enter_context(tc.tile_pool(name="wlate", bufs=1))
        attn_kv_cm = tc.tile_pool(name="attn_kv", bufs=1)
        attn_kv = attn_kv_cm.__enter__()
        kT = attn_kv.tile([128, NPAIR, T], bf)
        # v layout: [token-tile, head-pair, 130]: cols 0:64 even-head feats,
        # col 64 ones (even denominator), 65:129 odd feats, col 129 ones.
        v = attn_kv.tile([128, T // 128, NPAIR, 130], bf)
        nc.vector.memset(v, 1.0)

        with tc.tile_pool(name="ph_kv", bufs=1) as pkv, \
             tc.tile_pool(name="evw", bufs=3) as evw:
            mukv_row_t = pkv.tile([1, T], bf)
            rskv_row_t = pkv.tile([1, T], f32)
            rskv_cols = pkv.tile([128, T // 128], f32)
            stdkv_row_t = pkv.tile([1, T], bf) if flags["b1"] else None
            mukv_row = mukv_row_t[0:1, 0:T]
            rskv_row = rskv_row_t[0:1, 0:T]
            stdkv_row = stdkv_row_t[0:1, 0:T] if flags["b1"] else None
            xpT = pkv.tile([128, CT, T], bf)
            xpr = P["xpTbf"].rearrange("(a p) t -> p a t", p=128)
            nsk = pkv.tile([1, C], bf)
            nc.sync.dma_start(out=nsk[:], in_=P["nsk"][:, :])
            nsv = pkv.tile([1, C], bf)
            nc.sync.dma_start(out=nsv[:], in_=P["nsv"][:, :])
            if flags["b1"]:
                wbk = pkv.tile([1, C], bf)
                nc.sync.dma_start(out=wbk[:], in_=P["wbk"][:, :])
                wbv = pkv.tile([1, C], bf)
                nc.sync.dma_start(out=wbv[:], in_=P["wbv"][:, :])

            with tc.tile_pool(name="ph_k", bufs=1) as pk:
                # wk first (K needs all of it for its first psum group),
                # then x in token-chunk order; stats(c)+K(c) interleave so
                # each K block covers the next chunk's DMA and stat chain
                wk = pk.tile([128, CT, C], bf)
                dma_in3(wk, P["wk"].rearrange("(a p) f -> p a f", p=128))
                for ci in range(T // 512):
                    gs = slice(512 * ci, 512 * ci + 512)
                    for a in range(CT):
                        nc.sync.dma_start(out=xpT[:, a, gs],
                                          in_=xpr[:, a, gs])
                ln_stats(xpT, T, mukv_row, rskv_row, None, stdkv_row)
                for ci in range(T // 512):
                    qs = slice(512 * ci, 512 * ci + 512)
                    rsb = statw.tile([128, 512], f32, tag="rsb")
                    bcast_rows(rskv_row[:, qs], rsb[:], onesrow_32)
                    for ft in range(NPAIR):
                        fs = slice(128 * ft, 128 * ft + 128)
                        ps = ps_mm.tile([128, 512], f32, tag="mm")
                        for ct in range(CT):
                            nc.tensor.matmul(ps[:], wk[:, ct, fs], xpT[:, ct, qs],
                                             start=(ct == 0), stop=False)
                        nc.tensor.matmul(ps[:], nsk[0:1, fs], mukv_row[:, qs],
                                         start=False, stop=not flags["b1"])
                        if flags["b1"]:
                            nc.tensor.matmul(ps[:], wbk[0:1, fs],
                                             stdkv_row[:, qs],
                                             start=False, stop=True)
                        nc.vector.tensor_tensor(out=kT[:, ft, qs], in0=ps[:],
                                                in1=rsb[:], op=OP.mult)
                # column form of 1/std for the V evictions (DRAM bounce)
                scratch_d = dram.tile([T], f32, tag="st_dram")
                nc.sync.dma_start(out=scratch_d[:], in_=rskv_row)
                nc.sync.dma_start(
                    out=rskv_cols[:],
                    in_=scratch_d.rearrange("(j p) -> p j", p=128))

            with tc.tile_pool(name="ph_v", bufs=1) as pv:
                wv = pv.tile([128, CT, C], bf)
                dma_in3(wv, P["wv"].rearrange("(a p) f -> p a f", p=128))
                for tt in range(T // 128):
                    ts_ = slice(128 * tt, 128 * tt + 128)
                    for fc in range(2):
                        fs = slice(512 * fc, 512 * fc + 512)
                        ps = ps_mm.tile([128, 512], f32, tag="mm")
                        for ct in range(CT):
                            nc.tensor.matmul(ps[:], xpT[:, ct, ts_], wv[:, ct, fs],
                                             start=(ct == 0), stop=False)
                        nc.tensor.matmul(ps[:], mukv_row[:, ts_], nsv[0:1, fs],
                                         start=False, stop=not flags["b1"])
                        if flags["b1"]:
                            nc.tensor.matmul(ps[:], stdkv_row[:, ts_],
                                             wbv[0:1, fs], start=False, stop=True)
                        psr = ps[:].rearrange("p (a f) -> p a f", f=128)
                        prs = slice(4 * fc, 4 * fc + 4)
                        if flags["b1"]:
                            wbvb = evw.tile([128, 512], bf, tag="wbvb")
                            bcast_rows(wbv[0:1, fs], wbvb[:], onesrow_bf)
                            wbr = wbvb[:].rearrange("p (a f) -> p a f", f=128)
                            nc.vector.scalar_tensor_tensor(
                                out=v[:, tt, prs, 0:64], in0=psr[:, :, 0:64],
                                scalar=rskv_cols[:, tt:tt + 1],
                                in1=wbr[:, :, 0:64], op0=OP.mult, op1=OP.add)
                            nc.vector.scalar_tensor_tensor(
                                out=v[:, tt, prs, 65:129], in0=psr[:, :, 64:128],
                                scalar=rskv_cols[:, tt:tt + 1],
                                in1=wbr[:, :, 64:128], op0=OP.mult, op1=OP.add)
                        else:
                            nc.vector.tensor_scalar_mul(
                                v[:, tt, prs, 0:64], psr[:, :, 0:64],
                                rskv_cols[:, tt:tt + 1])
                            nc.vector.tensor_scalar_mul(
                                v[:, tt, prs, 65:129], psr[:, :, 64:128],
                                rskv_cols[:, tt:tt + 1])

        # Q matmuls (1/sqrt(D) folded into wq host-side)
        with tc.tile_pool(name="ph_q", bufs=1) as pq, \
             tc.tile_pool(name="evwq", bufs=3) as evwq:
            xTb = pq.tile([128, CT, TOWN], bf)
            dma_in3(xTb, P["xTbf"].rearrange("(a p) t -> p a t", p=128))
            wq = pq.tile([128, CT, C], bf)
            dma_in3(wq, P["wq"].rearrange("(a p) f -> p a f", p=128))
            nsq = pq.tile([1, C], bf)
            nc.sync.dma_start(out=nsq[:], in_=P["nsq"][:, :])
            if flags["b1"]:
                wbq = pq.tile([1, C], bf)
                nc.sync.dma_start(out=wbq[:], in_=P["wbq"][:, :])
            ln_stats(xTb, TOWN, muq_row, rsq_row, None, stdq_row)
            for ci in range(2):
                qs = slice(512 * ci, 512 * ci + 512)
                rsb = statw.tile([128, 512], f32, tag="rsb")
                bcast_rows(rsq_row[:, qs], rsb[:], onesrow_32)
                for ft in range(NPAIR):
                    fs = slice(128 * ft, 128 * ft + 128)
                    ps = ps_mm.tile([128, 512], f32, tag="mm")
                    for ct in range(CT):
                        nc.tensor.matmul(ps[:], wq[:, ct, fs], xTb[:, ct, qs],
                                         start=(ct == 0), stop=False)
                    nc.tensor.matmul(ps[:], nsq[0:1, fs], muq_row[:, qs],
                                     start=False, stop=not flags["b1"])
                    if flags["b1"]:
                        nc.tensor.matmul(ps[:], wbq[0:1, fs], stdq_row[:, qs],
                                         start=False, stop=True)
                    ev = evwq.tile([128, 512], bf, tag="ev")
                    nc.vector.tensor_tensor(out=ev[:], in0=ps[:],
                                            in1=rsb[:], op=OP.mult)
                    nc.sync.dma_start(out=qT_d[:, ft, qs], in_=ev[:])

        # late weights: prefetch during attention (DMA is idle there)
        wproj = wlate.tile([128, NPAIR, C], bf)
        dma_in3(wproj, P["wproj"].rearrange("(a p) f -> p a f", p=128))
        nsf1 = wlate.tile([1, FF], bf)
        nc.sync.dma_start(out=nsf1[:], in_=P["nsf1"][:, :])

        # ------------------------------------------------------------------
        # Phase 2: attention
        # ------------------------------------------------------------------
        with tc.tile_pool(name="amask", bufs=1) as pam, \
             tc.tile_pool(name="aload", bufs=4) as pal, \
             tc.tile_pool(name="awork", bufs=4) as paw, \
             tc.tile_pool(name="anorm", bufs=2) as pad, \
             tc.tile_pool(name="ps_s", bufs=2, space="PSUM") as ps_s, \
             tc.tile_pool(name="ps_y", bufs=1, space="PSUM") as ps_y:
            masksb = pam.tile([128, 16, 512], bf)
            dma_in3(masksb, P["masks"].rearrange("p (a w) -> p a w", w=512))

            def attn_norm_back(pr, ci, psYA, psYB, dde, ddo):
                """Deferred per-pair softmax normalization: PE broadcast of
                the reciprocal denominators + eviction to yT_d.  Emitted
                inside the NEXT pair's slot stream so the in-order PE never
                stalls on the reciprocal chain."""
                qs = slice(512 * ci, 512 * ci + 512)
                # broadcast the reciprocal denominators across partitions on
                # the otherwise-idle GPSIMD engine: no PE matmul, no PSUM
                # bank, no DVE staging copy in the normalization chain
                rbtA = pad.tile([64, 512], f32, tag="rbtA")
                nc.gpsimd.partition_broadcast(rbtA[:], dde[:])
                rbtB = pad.tile([64, 512], f32, tag="rbtB")
                nc.gpsimd.partition_broadcast(rbtB[:], ddo[:])
                yvE = pad.tile([64, 512], bf, tag="yvE")
                nc.vector.tensor_tensor(out=yvE[:], in0=psYA[0:64, :],
                                        in1=rbtA[:], op=OP.mult)
                yvO = pad.tile([64, 512], bf, tag="yvO")
                nc.vector.tensor_tensor(out=yvO[:], in0=psYB[0:64, :],
                                        in1=rbtB[:], op=OP.mult)
                nc.sync.dma_start(out=yT_d[0:64, pr, qs], in_=yvE[:])
                nc.sync.dma_start(out=yT_d[64:128, pr, qs], in_=yvO[:])

            iters = [(pr, ci) for pr in range(NPAIR) for ci in range(2)]
            qts = {}

            def load_qt(k):
                pr_, ci_ = iters[k]
                qt_ = pal.tile([128, 512], bf, tag="qt")
                nc.sync.dma_start(out=qt_[:],
                                  in_=qT_d[:, pr_,
                                           512 * ci_:512 * ci_ + 512])
                qts[k] = qt_

            # keep 3 q loads in flight so no slot ever waits on the DMA
            for k in range(3):
                load_qt(k)

            norm = None
            for it, (pr, ci) in enumerate(iters):
                    qs = slice(512 * ci, 512 * ci + 512)
                    nslot = SLOTS[ci]
                    qt = qts.pop(it)
                    if it + 3 < len(iters):
                        load_qt(it + 3)
                    psYA = ps_y.tile([128, 512], f32, tag="ya")
                    psYB = ps_y.tile([128, 512], f32, tag="yb")

                    def emit_sS_exp(s):
                        js = slice(128 * s, 128 * s + 128)
                        psS = ps_s.tile([128, 1024], f32, tag="s")
                        nc.tensor.matmul(psS[:, 0:512], kT[0:64, pr, js],
                                         qt[0:64, :], start=True, stop=True,
                                         tile_position=(0, 0))
                        nc.tensor.matmul(psS[:, 512:1024],
                                         kT[64:128, pr, js], qt[64:128, :],
                                         start=True, stop=True,
                                         tile_position=(64, 0))
                        pt = paw.tile([128, 1024], bf, tag="p")
                        nc.scalar.activation(pt[:], psS[:], AF.Exp)
                        if ci == 0 or s >= 8:
                            ms = masksb[:, s, :]
                            nc.vector.tensor_tensor(out=pt[:, 0:512],
                                                    in0=pt[:, 0:512], in1=ms,
                                                    op=OP.mult)
                            nc.vector.tensor_tensor(out=pt[:, 512:1024],
                                                    in0=pt[:, 512:1024],
                                                    in1=ms, op=OP.mult)
                        return pt

                    def emit_psY(s, pt):
                        st, sp = (s == 0), (s == nslot - 1)
                        nc.tensor.matmul(psYA[0:65, :], v[:, s, pr, 0:65],
                                         pt[:, 0:512], start=st, stop=sp,
                                         tile_position=(0, 0))
                        nc.tensor.matmul(psYB[0:65, :], v[:, s, pr, 65:130],
                                         pt[:, 512:1024], start=st, stop=sp,
                                         tile_position=(0, 0))

                    # software pipelining: psY(s) is emitted after sS(s+2),
                    # so its exp+mask inputs finished two slots ago and the
                    # in-order PE never waits on the Act/DVE chain.  The
                    # previous pair's deferred normalization lands right
                    # after sS(0), covering slot 0's exp latency.
                    pts = [emit_sS_exp(0)]
                    if norm is not None:
                        norm()
                    pts.append(emit_sS_exp(1))
                    for s in range(2, nslot):
                        pts.append(emit_sS_exp(s))
                        emit_psY(s - 2, pts[s - 2])
                    emit_psY(nslot - 2, pts[nslot - 2])
                    emit_psY(nslot - 1, pts[nslot - 1])

                    dde = pad.tile([1, 512], f32, tag="dde")
                    ddo = pad.tile([1, 512], f32, tag="ddo")
                    de_s = pad.tile([1, 512], f32, tag="de_s")
                    do_s = pad.tile([1, 512], f32, tag="do_s")
                    # custom-DVE ops mishandle base_partition != 0: stage the
                    # partition-64 denominator rows to partition 0 first.
                    nc.vector.tensor_copy(out=de_s[:], in_=psYA[64:65, :])
                    nc.vector.tensor_copy(out=do_s[:], in_=psYB[64:65, :])
                    nc.vector.reciprocal_approx_fast(out=dde[:], in_=de_s[:])
                    nc.vector.reciprocal_approx_fast(out=ddo[:], in_=do_s[:])
                    norm = (lambda pr=pr, ci=ci, psYA=psYA, psYB=psYB,
                            dde=dde, ddo=ddo:
                            attn_norm_back(pr, ci, psYA, psYB, dde, ddo))
            norm()

        if flags.get("dbg"):
            for a in range(NPAIR):
                nc.sync.dma_start(out=P["d_kT"][:, a, :], in_=kT[:, a, :])
            for a in range(T // 128):
                for pr_ in range(NPAIR):
                    nc.sync.dma_start(
                        out=P["d_v"][:, a, 128 * pr_:128 * pr_ + 64],
                        in_=v[:, a, pr_, 0:64])
                    nc.sync.dma_start(
                        out=P["d_v"][:, a, 128 * pr_ + 64:128 * pr_ + 128],
                        in_=v[:, a, pr_, 65:129])
        attn_kv_cm.__exit__(None, None, None)
        # ------------------------------------------------------------------
        # Phase 3: proj + residual, LN2, FF
        # ------------------------------------------------------------------
        xmid = es.enter_context(tc.tile_pool(name="xmid", bufs=1))
        xmT32 = xmid.tile([128, CT, TOWN], f32)
        geluT = xmid.tile([128, FT, TOWN], bf)
        sxm = xmid.tile([128, CT, TOWN], bf)

        pw_cm = tc.tile_pool(name="ph_w1", bufs=1)
        pw = pw_cm.__enter__()
        pp_cm = tc.tile_pool(name="ph_proj", bufs=1)
        pp = pp_cm.__enter__()
        pst_cm = tc.tile_pool(name="pstream", bufs=2)
        pst = pst_cm.__enter__()
        if True:
            # ff weights live in a pool OUTSIDE the proj pool so their
            # plane-DMAs can drain one-per-proj-iteration (the big transfers
            # never block the urgent per-ct y/x loads, and ff1/ff2 start
            # with their weights already resident)
            wff1 = pw.tile([128, CT, FF], bf)
            w1r = P["wff1"].rearrange("(a p) f -> p a f", p=128)
            w2r = P["wff2"].rearrange("(a p) f -> p a f", p=128)
            wdma = [(wff1, w1r, a) for a in range(CT)]
            if flags["gbias"]:
                gb = pw.tile([128, FT], f32)
                nc.sync.dma_start(out=gb[:], in_=P["geluBias"][:, :])
            if flags["bff2"]:
                b2row = pw.tile([1, C], bf)
                nc.sync.dma_start(out=b2row[:], in_=P["bf2row"][:, :])
            if flags["bproj"]:
                bprow = pw.tile([1, C], bf)
                nc.sync.dma_start(out=bprow[:], in_=P["bprow"][:, :])
            if flags["bproj"] or flags["bff2"]:
                onesrow = pw.tile([1, TOWN], bf)
                nc.vector.memset(onesrow, 1.0)
            xmbf = pp.tile([128, CT, TOWN], bf)

            ytfs = []
            for ci in range(2):
                qs = slice(512 * ci, 512 * ci + 512)
                ytf = pst.tile([128, NPAIR, 512], bf, tag="ytf")
                for ft in range(NPAIR):
                    nc.sync.dma_start(out=ytf[:, ft, :], in_=yT_d[:, ft, qs])
                ytfs.append(ytf)

            def proj_chunk(ci):
                qs = slice(512 * ci, 512 * ci + 512)
                ytf = ytfs[ci]
                for ct in range(CT):
                    if wdma:
                        dst, srcr, a = wdma.pop(0)
                        nc.sync.dma_start(out=dst[:, a, :], in_=srcr[:, a, :])
                    cs = slice(128 * ct, 128 * ct + 128)
                    ps = ps_mm.tile([128, 512], f32, tag="mm")
                    for ft in range(NPAIR):
                        nc.tensor.matmul(ps[:], wproj[:, ft, cs],
                                         ytf[:, ft, :], start=(ft == 0),
                                         stop=(ft == NPAIR - 1
                                               and not flags["bproj"]))
                    if flags["bproj"]:
                        nc.tensor.matmul(ps[:], bprow[0:1, cs],
                                         onesrow[0:1, qs],
                                         start=False, stop=True)
                    x32 = pst.tile([128, 512], f32, tag="x32")
                    nc.sync.dma_start(out=x32[:], in_=P["xT32"][cs, qs])
                    nc.vector.tensor_tensor(out=xmT32[:, ct, qs], in0=ps[:],
                                            in1=x32[:], op=OP.add)
                    nc.vector.tensor_copy(out=xmbf[:, ct, qs],
                                          in_=xmT32[:, ct, qs])

            def ln2_rows(ci):
                qs = slice(512 * ci, 512 * ci + 512)
                ln_stats(xmbf, 512, mu2_row_t[0:1, 0:TOWN], rs2_row,
                         None, None, row_off=512 * ci)
                nc.vector.tensor_tensor(out=mursf_row[:, qs],
                                        in0=rs2_row[:, qs],
                                        in1=mu2_row_t[0:1, qs], op=OP.mult)
                nc.vector.tensor_copy(out=murs2_row[:, qs],
                                      in_=mursf_row[:, qs])
                nc.vector.tensor_copy(out=rs2bf_row[:, qs],
                                      in_=rs2_row[:, qs])

            def sxm_chunk(ci):
                qs = slice(512 * ci, 512 * ci + 512)
                rb2 = statw.tile([128, 512], bf, tag="rb2")
                bcast_rows(rs2bf_row[:, qs], rb2[:], onesrow_bf)
                for ct in range(CT):
                    nc.vector.tensor_tensor(out=sxm[:, ct, qs],
                                            in0=xmbf[:, ct, qs], in1=rb2[:],
                                            op=OP.mult)


            def ff1_chunk(ci, wq2):
                qs = slice(512 * ci, 512 * ci + 512)
                for ft in range(FT):
                    fs = slice(128 * ft, 128 * ft + 128)
                    if wq2:
                        dst, srcr, a = wq2.pop(0)
                        nc.sync.dma_start(out=dst[:, a, :], in_=srcr[:, a, :])
                    ps = ps_mm.tile([128, 512], f32, tag="mm")
                    for ct in range(CT):
                        nc.tensor.matmul(ps[:], wff1[:, ct, fs],
                                         sxm[:, ct, qs],
                                         start=(ct == 0), stop=False)
                    nc.tensor.matmul(ps[:], nsf1[0:1, fs], murs2_row[:, qs],
                                     start=False, stop=True)
                    bias = gb[:, ft:ft + 1] if flags["gbias"] else 0.0
                    nc.scalar.activation(geluT[:, ft, qs], ps[:], AF.Gelu,
                                         bias=bias)

            # per-chunk order: chunk 0's LN2 stat chain (Act sqrt + DVE
            # recip) hides under proj(1), chunk 1's under ff1(0); the Act
            # table sequence is sqrt,sqrt,gelu,gelu (1 switch)
            proj_chunk(0)
            ln2_rows(0)
            proj_chunk(1)
            sxm_chunk(0)
            ln2_rows(1)
            ff1_chunk(0, None)
            sxm_chunk(1)

        pst_cm.__exit__(None, None, None)
        pp_cm.__exit__(None, None, None)
        # wff2 lands in the space ph_proj/pstream just freed and streams in
        # under the ff1 matmuls
        with tc.tile_pool(name="ph_w2", bufs=1) as pw2, \
             tc.tile_pool(name="outp", bufs=2) as po:
            wff2 = pw2.tile([128, FT, C], bf)
            wdma2 = [(wff2, w2r, a) for a in range(FT)]

            def ff2_out(ci):
                qs = slice(512 * ci, 512 * ci + 512)
                for ct in range(CT):
                    cs = slice(128 * ct, 128 * ct + 128)
                    ps = ps_mm.tile([128, 512], f32, tag="mm")
                    for ft in range(FT):
                        nc.tensor.matmul(ps[:], wff2[:, ft, cs],
                                         geluT[:, ft, qs],
                                         start=(ft == 0),
                                         stop=(ft == FT - 1
                                               and not flags["bff2"]))
                    if flags["bff2"]:
                        nc.tensor.matmul(ps[:], b2row[0:1, cs],
                                         onesrow[0:1, qs],
                                         start=False, stop=True)
                    ot = po.tile([128, 512], f32, tag="ot")
                    nc.vector.tensor_tensor(out=ot[:], in0=ps[:],
                                            in1=xmT32[:, ct, qs], op=OP.add)
                    nc.sync.dma_start(out=P["outT"][cs, qs], in_=ot[:])

            ff1_chunk(1, wdma2)
            ff2_out(0)
            ff2_out(1)
        pw_cm.__exit__(None, None, None)

        if flags.get("dbg"):
            for a in range(NPAIR):
                nc.sync.dma_start(out=P["d_qT"][:, a, :], in_=qT_d[:, a, :])
                nc.sync.dma_start(out=P["d_yT"][:, a, :], in_=yT_d[:, a, :])
            for a in range(CT):
                nc.sync.dma_start(out=P["d_xm"][128 * a:128 * a + 128, :],
                                  in_=xmT32[:, a, :])
            nc.sync.dma_start(out=P["d_rows"][0:1, 0:T], in_=rskv_row)
            nc.sync.dma_start(out=P["d_rows"][1:2, 0:TOWN], in_=rsq_row)
            nc.sync.dma_start(out=P["d_rows"][2:3, 0:TOWN], in_=rs2_row)
            nc.sync.dma_start(
                out=P["d_rows"][3:4, 0:T].rearrange("o (j p) -> o p j", p=128),
                in_=rskv_cols[:, :])


_WAIT_LIMITS = {
    # walrus codegen encodes sync waits inside the 64B instruction; compute
    # ISA structs only have room for one.  Hoist the overflow onto
    # same-engine NoOps (the sequencer processes waits in program order, so
    # semantics are identical).
    "TensorTensor": 1, "TensorScalarPtr": 1, "Activation": 1, "Matmult": 1,
    "Ldweights": 1, "TensorReduce": 1, "Memset": 1, "TensorCopy": 1,
    "ISA": 1, "Iota": 1, "Reciprocal": 1, "CustomDveAnt": 1, "NoOp": 1,
    "EventSemaphore": 1, "Drain": 1, "DMACopy": 1,
}
_nop_ctr = [0]


def _split_waits(nc):
    import concourse.mybir as mb
    for f in nc.m.functions:
        for bb in f.blocks:
            out = []
            for inst in bb.instructions:
                si = inst.sync_info
                lim = _WAIT_LIMITS.get(getattr(inst, "opcode", None), None)
                if (si is not None and si.on_wait and lim is not None
                        and len(si.on_wait) > lim):
                    waits = list(si.on_wait)
                    extra, keep = waits[:-lim], waits[-lim:]
                    while extra:
                        chunk, extra = extra[:1], extra[1:]
                        _nop_ctr[0] += 1
                        nop = mb.InstEventSemaphore(
                            name=f"I-waitnop-{_nop_ctr[0]}", ins=[], outs=[])
                        nop.engine = inst.engine
                        nop.sync_info = mb.SyncInfo(on_wait=chunk, on_update=[])
                        out.append(nop)
                    inst.sync_info = mb.SyncInfo(on_wait=keep,
                                                 on_update=si.on_update)
                out.append(inst)
            bb.instructions[:] = out


LAST_NC = None
LAST_INMAPS = None


def bench(iters=30):
    """Repeatedly execute the compiled NEFF with device-resident inputs and
    return the min per-iteration wall time in ns (upper bound on HW exec:
    includes PJRT dispatch + axon tunnel overhead, amortized)."""
    import time

    import jax
    import concourse.mybir as mb
    from concourse.bass2jax import (_bass_exec_p, install_neuronx_cc_hook,
                                    Mesh, PartitionSpec, shard_map,
                                    partition_id_tensor)
    from jax.sharding import NamedSharding

    nc, in_maps = LAST_NC, LAST_INMAPS
    assert nc is not None
    install_neuronx_cc_hook()
    pname = nc.partition_id_tensor.name if nc.partition_id_tensor else None
    in_names, out_names, out_avals, zero_outs = [], [], [], []
    for alloc in nc.m.functions[0].allocations:
        if not isinstance(alloc, mb.MemoryLocationSet):
            continue
        name = alloc.memorylocations[0].name
        if alloc.kind == "ExternalInput":
            if name != pname:
                in_names.append(name)
        elif alloc.kind == "ExternalOutput":
            out_names.append(name)
            shape = tuple(alloc.tensor_shape)
            dtype = mb.dt.np(alloc.dtype)
            out_avals.append(jax.core.ShapedArray(shape, dtype))
            zero_outs.append(np.zeros(shape, dtype))
    n_params = len(in_names)
    all_names = in_names + out_names
    if pname is not None:
        all_names = all_names + [pname]

    def _body(*args):
        operands = list(args)
        if pname is not None:
            operands.append(partition_id_tensor())
        return tuple(_bass_exec_p.bind(
            *operands, out_avals=tuple(out_avals), in_names=tuple(all_names),
            out_names=tuple(out_names), lowering_input_output_aliases=(),
            sim_require_finite=True, sim_require_nnan=True, nc=nc))

    devices = jax.devices()[:NCORE]
    mesh = Mesh(np.asarray(devices), ("core",))
    spec = PartitionSpec("core")
    sharded = jax.jit(
        shard_map(_body, mesh=mesh, in_specs=(spec,) * (n_params + len(out_names)),
                  out_specs=(spec,) * len(out_names), check_rep=False),
        keep_unused=True)
    sh = NamedSharding(mesh, spec)
    dev_in = [jax.device_put(
        np.concatenate([np.asarray(in_maps[c][nm]) for c in range(NCORE)], 0), sh)
        for nm in in_names]
    dev_in += [jax.device_put(
        np.concatenate([z] * NCORE, 0), sh) for z in zero_outs]
    out = sharded(*dev_in)
    jax.block_until_ready(out)          # compile + warm
    times = []
    for _ in range(iters):
        t0 = time.perf_counter()
        out = sharded(*dev_in)
        jax.block_until_ready(out)
        times.append(time.perf_counter() - t0)
    times.sort()
    return {"min_ns": int(times[0] * 1e9),
            "p50_ns": int(times[len(times) // 2] * 1e9),
            "times_ms": [round(t * 1e3, 3) for t in times[:5]]}


def bench_amortized(n_lo=8, n_hi=40, reps=8):
    """Per-iteration device time via pipelined dispatch: enqueue n back-to-back
    executions of the compiled NEFF (device-resident inputs), block once at the
    end.  The axon/PJRT dispatch pipeline overlaps RPC latency with device
    execution, so T(n_hi) - T(n_lo) isolates n_hi - n_lo real executions:
    per_iter = (T_hi - T_lo) / (n_hi - n_lo).  Each T is min over `reps`."""
    import time

    import jax
    import concourse.mybir as mb
    from concourse.bass2jax import (_bass_exec_p, install_neuronx_cc_hook,
                                    Mesh, PartitionSpec, shard_map,
                                    partition_id_tensor)
    from jax.sharding import NamedSharding

    nc, in_maps = LAST_NC, LAST_INMAPS
    assert nc is not None
    install_neuronx_cc_hook()
    pname = nc.partition_id_tensor.name if nc.partition_id_tensor else None
    in_names, out_names, out_avals, zero_outs = [], [], [], []
    for alloc in nc.m.functions[0].allocations:
        if not isinstance(alloc, mb.MemoryLocationSet):
            continue
        name = alloc.memorylocations[0].name
        if alloc.kind == "ExternalInput":
            if name != pname:
                in_names.append(name)
        elif alloc.kind == "ExternalOutput":
            out_names.append(name)
            shape = tuple(alloc.tensor_shape)
            dtype = mb.dt.np(alloc.dtype)
            out_avals.append(jax.core.ShapedArray(shape, dtype))
            zero_outs.append(np.zeros(shape, dtype))
    n_params = len(in_names)
    all_names = in_names + out_names
    if pname is not None:
        all_names = all_names + [pname]

    def _body(*args):
        operands = list(args)
        if pname is not None:
            operands.append(partition_id_tensor())
        return tuple(_bass_exec_p.bind(
            *operands, out_avals=tuple(out_avals), in_names=tuple(all_names),
            out_names=tuple(out_names), lowering_input_output_aliases=(),
            sim_require_finite=True, sim_require_nnan=True, nc=nc))

    devices = jax.devices()[:NCORE]
    mesh = Mesh(np.asarray(devices), ("core",))
    spec = PartitionSpec("core")
    sharded = jax.jit(
        shard_map(_body, mesh=mesh, in_specs=(spec,) * (n_params + len(out_names)),
                  out_specs=(spec,) * len(out_names), check_rep=False),
        keep_unused=True)
    sh = NamedSharding(mesh, spec)
    dev_in = [jax.device_put(
        np.concatenate([np.asarray(in_maps[c][nm]) for c in range(NCORE)], 0), sh)
        for nm in in_names]
    dev_in += [jax.device_put(
        np.concatenate([z] * NCORE, 0), sh) for z in zero_outs]
    jax.block_until_ready(sharded(*dev_in))      # compile + warm

    def chain_time(n):
        best = float("inf")
        for _ in range(reps):
            t0 = time.perf_counter()
            outs = [sharded(*dev_in) for _ in range(n)]
            jax.block_until_ready(outs)
            best = min(best, time.perf_counter() - t0)
        return best

    chain_time(n_lo)                              # extra warm for the pipeline
    t_lo = chain_time(n_lo)
    t_hi = chain_time(n_hi)
    per_iter = (t_hi - t_lo) / (n_hi - n_lo)
    return {"per_iter_ns": max(int(per_iter * 1e9), 1),
            "t_lo_ms": round(t_lo * 1e3, 3),
            "t_hi_ms": round(t_hi * 1e3, 3)}


def bench_chain(n_lo=2, n_hi=18, reps=12):
    """Ground-truth device timing: one jitted program executes the NEFF n
    times back-to-back (outT threaded into xT32 to serialize); the timing
    difference between n_hi and n_lo cancels the dispatch/tunnel overhead."""
    import time

    import jax
    import concourse.mybir as mb
    from concourse.bass2jax import (_bass_exec_p, install_neuronx_cc_hook,
                                    Mesh, PartitionSpec, shard_map,
                                    partition_id_tensor)
    from jax.sharding import NamedSharding

    nc, in_maps = LAST_NC, LAST_INMAPS
    assert nc is not None
    install_neuronx_cc_hook()
    pname = nc.partition_id_tensor.name if nc.partition_id_tensor else None
    in_names, out_names, out_avals, zero_outs = [], [], [], []
    for alloc in nc.m.functions[0].allocations:
        if not isinstance(alloc, mb.MemoryLocationSet):
            continue
        name = alloc.memorylocations[0].name
        if alloc.kind == "ExternalInput":
            if name != pname:
                in_names.append(name)
        elif alloc.kind == "ExternalOutput":
            out_names.append(name)
            shape = tuple(alloc.tensor_shape)
            dtype = mb.dt.np(alloc.dtype)
            out_avals.append(jax.core.ShapedArray(shape, dtype))
            zero_outs.append(np.zeros(shape, dtype))
    n_params = len(in_names)
    all_names = in_names + out_names + ([pname] if pname else [])
    x_idx = in_names.index("xT32")
    o_idx = out_names.index("outT")

    def mk_body(n):
        def _body(*args):
            ins = list(args[:n_params])
            zouts = list(args[n_params:])
            for _ in range(n):
                operands = ins + zouts
                if pname is not None:
                    operands.append(partition_id_tensor())
                outs = _bass_exec_p.bind(
                    *operands, out_avals=tuple(out_avals),
                    in_names=tuple(all_names), out_names=tuple(out_names),
                    lowering_input_output_aliases=(),
                    sim_require_finite=True, sim_require_nnan=True, nc=nc)
                ins[x_idx] = outs[o_idx]
            return tuple(outs)
        return _body

    devices = jax.devices()[:NCORE]
    mesh = Mesh(np.asarray(devices), ("core",))
    spec = PartitionSpec("core")
    sh = NamedSharding(mesh, spec)
    dev_in = [jax.device_put(
        np.concatenate([np.asarray(in_maps[c][nm]) for c in range(NCORE)], 0),
        sh) for nm in in_names]
    dev_in += [jax.device_put(np.concatenate([z] * NCORE, 0), sh)
               for z in zero_outs]

    res = {}
    for n in (n_lo, n_hi):
        f = jax.jit(shard_map(mk_body(n), mesh=mesh,
                              in_specs=(spec,) * len(dev_in),
                              out_specs=(spec,) * len(out_names),
                              check_rep=False), keep_unused=True)
        jax.block_until_ready(f(*dev_in))       # compile + warm
        ts = []
        for _ in range(reps):
            t0 = time.perf_counter()
            jax.block_until_ready(f(*dev_in))
            ts.append(time.perf_counter() - t0)
        ts.sort()
        res[n] = ts[0]
    per_iter = (res[n_hi] - res[n_lo]) / (n_hi - n_lo)
    return {"per_iter_ns": int(per_iter * 1e9),
            "t_lo_ms": round(res[n_lo] * 1e3, 2),
            "t_hi_ms": round(res[n_hi] * 1e3, 2)}


def _build_nc(flags):
    nc = bacc.Bacc("TRN2", target_bir_lowering=False, debug=False,
                   num_devices=NCORE)
    P = {}

    def inp(name, shape, d):
        P[name] = nc.declare_dram_parameter(name, list(shape), d, isOutput=False)

    inp("xT32", (C, TOWN), f32)
    inp("xTbf", (C, TOWN), bf)
    inp("xpTbf", (C, T), bf)
    inp("wq", (C, C), bf)
    inp("wk", (C, C), bf)
    inp("wv", (C, C), bf)
    inp("wproj", (C, C), bf)
    inp("wff1", (C, FF), bf)
    inp("wff2", (FF, C), bf)
    inp("nsq", (1, C), bf)
    inp("nsk", (1, C), bf)
    inp("nsv", (1, C), bf)
    inp("nsf1", (1, FF), bf)
    inp("masks", (128, 16 * 512), bf)
    if flags["b1"]:
        inp("wbq", (1, C), bf)
        inp("wbk", (1, C), bf)
        inp("wbv", (1, C), bf)
    if flags["bproj"]:
        inp("bprow", (1, C), bf)
    if flags["gbias"]:
        inp("geluBias", (128, FT), f32)
    if flags["bff2"]:
        inp("bf2row", (1, C), bf)
    P["outT"] = nc.declare_dram_parameter("outT", [C, TOWN], f32, isOutput=True)
    if flags.get("dbg"):
        for nm, shape, d in [("d_kT", [128, NPAIR, T], bf),
                             ("d_qT", [128, NPAIR, TOWN], bf),
                             ("d_v", [128, T // 128, C], bf),
                             ("d_yT", [128, NPAIR, TOWN], bf),
                             ("d_xm", [C, TOWN], f32),
                             ("d_rows", [8, T], f32),
                             ("d_S", [8, 128, 1024], f32),
                             ("d_P", [8, 128, 1024], bf),
                             ("d_ypre", [128, 1024], f32)]:
            P[nm] = nc.declare_dram_parameter(nm, shape, d, isOutput=True)

    with tile.TileContext(nc, pool_alloc_mode="queue") as tc:
        _emit(tc, P, flags)
    nc.compile()
    return nc


# --------------------------------------------------------------------------
# host side
# --------------------------------------------------------------------------

def _own_rows(half):
    a, b = BLOCKS[half]
    return np.concatenate([np.arange(512 * a, 512 * a + 512),
                           np.arange(512 * b, 512 * b + 512)])


def _mask_pack(half):
    """[128, 16*512] bf16; col-block s = keep-mask for key-tile slot s."""
    out = np.ones((128, 16 * 512), dtype=F32)
    jj = np.arange(128)[:, None]
    qq = np.arange(512)[None, :]
    a, b = BLOCKS[half]
    for s in range(8):
        out[:, 512 * s:512 * s + 512] = (128 * s + jj) <= (512 * a + qq)
    for s in range(8, 16):
        out[:, 512 * s:512 * s + 512] = (128 * s + jj) <= (512 * b + qq)
    return out.astype(BF16)


def kernel(**inputs):
    global LAST_RESULT
    ins = {k: np.asarray(v) for k, v in inputs.items()}
    x = ins["x"].astype(F32)
    perm = np.asarray(ins["perm"]).astype(np.int64)
    Wqkv, Wproj = ins["Wqkv"].astype(F32), ins["Wproj"].astype(F32)
    bproj = ins["bproj"].astype(F32)
    g1, b1 = ins["ln1_g"].astype(F32), ins["ln1_b"].astype(F32)
    g2, b2 = ins["ln2_g"].astype(F32), ins["ln2_b"].astype(F32)
    Wff1, bff1 = ins["Wff1"].astype(F32), ins["bff1"].astype(F32)
    Wff2, bff2 = ins["Wff2"].astype(F32), ins["bff2"].astype(F32)

    sigma = np.argsort(perm)
    sc = 1.0 / np.sqrt(D)

    wq_f = Wqkv[:, :C] * g1[:, None] * sc
    wk_f = Wqkv[:, C:2 * C] * g1[:, None]
    wv_f = Wqkv[:, 2 * C:] * g1[:, None]
    wf1_f = Wff1 * g2[:, None]

    flags = {
        "b1": bool(np.any(b1 != 0.0)),
        "bproj": bool(np.any(bproj != 0.0)),
        "gbias": bool(np.any(bff1 != 0.0) or np.any(b2 != 0.0)),
        "bff2": bool(np.any(bff2 != 0.0)),
        "dbg": bool(os.environ.get("KDBG")),
    }

    shared = {
        "wq": wq_f.astype(BF16), "wk": wk_f.astype(BF16),
        "wv": wv_f.astype(BF16), "wproj": Wproj.astype(BF16),
        "wff1": wf1_f.astype(BF16), "wff2": Wff2.astype(BF16),
        "nsq": (-wq_f.sum(0))[None, :].astype(BF16),
        "nsk": (-wk_f.sum(0))[None, :].astype(BF16),
        "nsv": (-wv_f.sum(0))[None, :].astype(BF16),
        "nsf1": (-wf1_f.sum(0))[None, :].astype(BF16),
    }
    if flags["b1"]:
        shared["wbq"] = (b1 @ Wqkv[:, :C] * sc)[None, :].astype(BF16)
        shared["wbk"] = (b1 @ Wqkv[:, C:2 * C])[None, :].astype(BF16)
        shared["wbv"] = (b1 @ Wqkv[:, 2 * C:])[None, :].astype(BF16)
    if flags["bproj"]:
        shared["bprow"] = bproj[None, :].astype(BF16)
    if flags["gbias"]:
        gb = (bff1 + b2 @ Wff1).astype(F32)           # [FF]
        shared["geluBias"] = np.ascontiguousarray(
            gb.reshape(FT, 128).T).astype(F32)        # [128, FT]
    if flags["bff2"]:
        shared["bf2row"] = bff2[None, :].astype(BF16)

    in_maps = []
    for c in range(NCORE):
        b, half = c // 2, c % 2
        rows_ = _own_rows(half)
        xb = x[b]
        xq = xb[rows_]
        m = dict(shared)
        m["xT32"] = np.ascontiguousarray(xq.T)
        m["xTbf"] = m["xT32"].astype(BF16)
        m["xpTbf"] = np.ascontiguousarray(xb[sigma].T).astype(BF16)
        m["masks"] = _mask_pack(half)
        in_maps.append(m)

    global LAST_NC, LAST_INMAPS
    nc = _build_nc(flags)
    LAST_NC, LAST_INMAPS = nc, in_maps
    res = run_bass_kernel_spmd(nc, in_maps, list(range(NCORE)))
    LAST_RESULT = res

    out = np.empty((B, T, C), dtype=F32)
    for c in range(NCORE):
        b, half = c // 2, c % 2
        out[b, _own_rows(half)] = res.results[c]["outT"].T
    return out


if __name__ == "__main__":
    rng = np.random.default_rng(0)
    demo = {
        "x": rng.standard_normal((B, T, C), dtype=F32),
        "perm": rng.permutation(T).astype(np.int32),
        "Wqkv": rng.standard_normal((C, 3 * C), dtype=F32) / 32,
        "Wproj": rng.standard_normal((C, C), dtype=F32) / 32,
        "bproj": np.zeros(C, F32),
        "ln1_g": np.ones(C, F32), "ln1_b": np.zeros(C, F32),
        "ln2_g": np.ones(C, F32), "ln2_b": np.zeros(C, F32),
        "Wff1": rng.standard_normal((C, FF), dtype=F32) / 32,
        "bff1": np.zeros(FF, F32),
        "Wff2": rng.standard_normal((FF, C), dtype=F32) / 45,
        "bff2": np.zeros(C, F32),
    }
    o = kernel(**demo)
    print("ok", o.shape, o.dtype)



# revision 21
# speedup vs baseline: 6.1522x; 6.1522x over previous
"""Trainium2 Bass kernel for a dense transformer block (B=4, T=2048, C=1024,
H=16, FF=2048) with a random-permuted causal mask.

Strategy (8 NeuronCores, SPMD, collective-free):
  - 2 cores per batch; each core owns 1024 query rows = two global 512-row
    q-blocks, picked so causal work balances: half0 -> blocks {0,3},
    half1 -> blocks {1,2}.  Uniform program: block A runs 8 key-tile slots,
    block B 16 slots; per-core mask data zeroes the slots a core doesn't need.
  - Keys are processed in perm-sorted order (sigma = argsort(perm)), which
    turns the permuted mask into a standard causal mask -> block skipping.
  - The residual stream is kept feature-major (transposed) on chip so the
    layernorms fold into the matmuls:  LN gain g folds into weights host-side,
    the mean folds in as an extra K=1 contraction row, and 1/std is applied
    multiplicatively at PSUM eviction.  No on-chip transposes at all.
  - Each core recomputes K/V for its full batch from a host-permuted,
    host-transposed copy of x (no inter-core communication anywhere).
  - q/k/v/y live in DRAM between phases (SBUF is the scarce resource).
Output is returned feature-major per core and transposed on host.
"""

import os
import sys
from contextlib import ExitStack

import numpy as np

for _p in ("/opt/trn_rl_repo",):
    if os.path.isdir(_p) and _p not in sys.path:
        sys.path.insert(0, _p)

import ml_dtypes

import concourse.bass as bass
import concourse.mybir as mybir
import concourse.tile as tile
from concourse import bacc
from concourse.bass_utils import run_bass_kernel_spmd

BF16 = ml_dtypes.bfloat16
F32 = np.float32

B, T, C, H, D, FF = 4, 2048, 1024, 16, 64, 2048
EPS = 1e-5
NCORE = 8
CT = C // 128          # 8 contraction tiles over C
FT = FF // 128         # 16 tiles over FF
NPAIR = H // 2         # 8 head pairs
TOWN = 1024            # query tokens owned per core
SLOTS = (8, 16)        # key-tile slots for q-block A / q-block B
BLOCKS = {0: (0, 3), 1: (1, 2)}   # half -> (global q-block A, B)

LAST_RESULT = None     # BassKernelResults of the last run (for test harness)

f32 = mybir.dt.float32
bf = mybir.dt.bfloat16
AF = mybir.ActivationFunctionType
OP = mybir.AluOpType


# --------------------------------------------------------------------------
# program builder
# --------------------------------------------------------------------------

def _emit(tc, P, flags):
    nc = tc.nc
    es = ExitStack()

    with es:
        const = es.enter_context(tc.tile_pool(name="const", bufs=1))
        dram = es.enter_context(tc.tile_pool(name="dram", bufs=1, space="DRAM"))
        ps_mm = es.enter_context(tc.tile_pool(name="ps_mm", bufs=2, space="PSUM"))
        statw = es.enter_context(tc.tile_pool(name="statw", bufs=2))
        statr = es.enter_context(tc.tile_pool(name="statr", bufs=1))

        onescol = const.tile([128, 1], bf)
        nc.vector.memset(onescol, 1.0)
        epscol = const.tile([128, 1], f32)
        nc.vector.memset(epscol, EPS)
        onesrow_bf = const.tile([1, 128], bf)
        nc.vector.memset(onesrow_bf, 1.0)
        onesrow_32 = const.tile([1, 128], f32)
        nc.vector.memset(onesrow_32, 1.0)

        def bcast_rows(row_ap, out_tile, ones_row):
            """out_tile[p, :] = row_ap[0, :] for all p, on the idle GPSIMD
            engine (no PE matmul, no PSUM bank, no DVE staging copy)."""
            nc.gpsimd.partition_broadcast(out_tile, row_ap)

        def dma_in3(dst, dram_ap):
            """Split a [128, n, W] load into per-plane DMAs so no consumer
            needs more sync waits than one instruction can encode."""
            n = dst.shape[1]
            for a in range(n):
                nc.sync.dma_start(out=dst[:, a, :], in_=dram_ap[:, a, :])

        # per-token scalar rows.  Rows that feed matmuls (as K=1 contraction
        # rows) must sit at base partition 0 -> own [1, N] tiles.  Rows only
        # read by DMA/DVE are packed into shared [128, T] tiles.
        muq_row_t = const.tile([1, TOWN], bf)
        murs2_row_t = const.tile([1, TOWN], bf)
        rs2bf_row_t = const.tile([1, TOWN], bf)
        mu2_row_t = const.tile([1, TOWN], bf)
        rsq_row_t = const.tile([1, TOWN], f32)
        rs2_row_t = const.tile([1, TOWN], f32)
        mursf_row_t = const.tile([1, TOWN], f32)
        if flags["b1"]:
            stdq_row_t = const.tile([1, TOWN], bf)

        # DRAM bounce tensors (qT/yT only; kT/v stay SBUF-resident)
        qT_d = dram.tile([128, NPAIR, TOWN], bf)
        yT_d = dram.tile([128, NPAIR, TOWN], bf)

        def ln_stats(xT, ntok, murow, rsrow, rs_cols, stdrow, row_off=0):
            """Feature-major LN stats over [row_off, row_off+ntok).  xT is the
            FULL [128, CT, *] bf16 tensor; murow (bf16) / rsrow (f32) /
            stdrow (bf16) are full-width [1, *] row APs; rs_cols optionally
            gets the [128, ntok//128] f32 column form (token t ->
            [t % 128, t // 128]) via a DRAM-bounce reshape."""
            nch = ntok // 512
            for ci in range(nch):
                qs = slice(row_off + 512 * ci, row_off + 512 * ci + 512)
                ps = ps_mm.tile([128, 512], f32, tag="mm")
                for ct in range(CT):
                    xs = xT[:, ct, qs]
                    sq = statw.tile([128, 512], bf, tag="st_sq")
                    nc.vector.tensor_tensor(out=sq[:], in0=xs, in1=xs, op=OP.mult)
                    nc.tensor.matmul(ps[0:1, :], onescol[:], xs,
                                     start=(ct == 0), stop=(ct == CT - 1),
                                     tile_position=(0, 0))
                    nc.tensor.matmul(ps[32:33, :], onescol[:], sq[:],
                                     start=(ct == 0), stop=(ct == CT - 1),
                                     tile_position=(0, 32))
                # row-form stats for this 512-token chunk
                muf = statr.tile([1, 512], f32, tag="st_muf")
                nc.scalar.mul(muf[:], ps[0:1, :], 1.0 / C)
                musq = statr.tile([1, 512], f32, tag="st_musq")
                nc.vector.tensor_tensor(out=musq[:], in0=muf[:], in1=muf[:],
                                        op=OP.mult)
                var = statr.tile([1, 512], f32, tag="st_var")
                nc.vector.scalar_tensor_tensor(out=var[:], in0=ps[32:33, :],
                                               scalar=1.0 / C, in1=musq[:],
                                               op0=OP.mult, op1=OP.subtract)
                std = statr.tile([1, 512], f32, tag="st_std")
                nc.scalar.activation(std[:], var[:], AF.Sqrt,
                                     bias=epscol[0:1, :])
                scr = statr.tile([1, 512], f32, tag="st_scr")
                nc.vector.reciprocal_approx_accurate(out=rsrow[:, qs],
                                                     in_=std[:], scratch=scr[:])
                nc.vector.tensor_copy(out=murow[:, qs], in_=muf[:])
                if stdrow is not None:
                    nc.vector.tensor_copy(out=stdrow[:, qs], in_=std[:])
            if rs_cols is not None:
                scratch_d = dram.tile([ntok], f32, tag="st_dram")
                nc.sync.dma_start(out=scratch_d[:], in_=rsrow[:, 0:ntok])
                nc.sync.dma_start(
                    out=rs_cols,
                    in_=scratch_d.rearrange("(j p) -> p j", p=128))

        muq_row = muq_row_t[0:1, 0:TOWN]
        murs2_row = murs2_row_t[0:1, 0:TOWN]
        rs2bf_row = rs2bf_row_t[0:1, 0:TOWN]
        stdq_row = stdq_row_t[0:1, 0:TOWN] if flags["b1"] else None
        rsq_row = rsq_row_t[0:1, 0:TOWN]
        rs2_row = rs2_row_t[0:1, 0:TOWN]
        mursf_row = mursf_row_t[0:1, 0:TOWN]

        # ------------------------------------------------------------------
        # Phase 1: stats + K + V from x_perm^T, then Q from x^T.
        # The Q-phase inputs are prefetched up front (DMA is otherwise idle
        # during the K/V matmuls).
        # ------------------------------------------------------------------
        wlate = es.enter_context(tc.tile_pool(name="wlate", bufs=1))
        attn_kv_cm = tc.tile_pool(name="attn_kv", bufs=1)
        attn_kv = attn_kv_cm.__enter__()
        kT = attn_kv.tile([128, NPAIR, T], bf)
        # v layout: [token-tile, head-pair, 130]: cols 0:64 even-head feats,
        # col 64 ones (even denominator), 65:129 odd feats, col 129 ones.
        v = attn_kv.tile([128, T // 128, NPAIR, 130], bf)
        nc.vector.memset(v, 1.0)

        with tc.tile_pool(name="ph_kv", bufs=1) as pkv, \
             tc.tile_pool(name="evw", bufs=3) as evw:
            mukv_row_t = pkv.tile([1, T], bf)
            rskv_row_t = pkv.tile([1, T], f32)
            rskv_cols = pkv.tile([128, T // 128], f32)
            stdkv_row_t = pkv.tile([1, T], bf) if flags["b1"] else None
            mukv_row = mukv_row_t[0:1, 0:T]
            rskv_row = rskv_row_t[0:1, 0:T]
            stdkv_row = stdkv_row_t[0:1, 0:T] if flags["b1"] else None
            xpT = pkv.tile([128, CT, T], bf)
            xpr = P["xpTbf"].rearrange("(a p) t -> p a t", p=128)
            nsk = pkv.tile([1, C], bf)
            nc.sync.dma_start(out=nsk[:], in_=P["nsk"][:, :])
            nsv = pkv.tile([1, C], bf)
            nc.sync.dma_start(out=nsv[:], in_=P["nsv"][:, :])
            if flags["b1"]:
                wbk = pkv.tile([1, C], bf)
                nc.sync.dma_start(out=wbk[:], in_=P["wbk"][:, :])
                wbv = pkv.tile([1, C], bf)
                nc.sync.dma_start(out=wbv[:], in_=P["wbv"][:, :])

            with tc.tile_pool(name="ph_k", bufs=1) as pk:
                # wk first (K needs all of it for its first psum group),
                # then x in token-chunk order; stats(c)+K(c) interleave so
                # each K block covers the next chunk's DMA and stat chain
                wk = pk.tile([128, CT, C], bf)
                dma_in3(wk, P["wk"].rearrange("(a p) f -> p a f", p=128))
                for ci in range(T // 512):
                    gs = slice(512 * ci, 512 * ci + 512)
                    for a in range(CT):
                        nc.sync.dma_start(out=xpT[:, a, gs],
                                          in_=xpr[:, a, gs])
                ln_stats(xpT, T, mukv_row, rskv_row, None, stdkv_row)
                for ci in range(T // 512):
                    qs = slice(512 * ci, 512 * ci + 512)
                    rsb = statw.tile([128, 512], f32, tag="rsb")
                    bcast_rows(rskv_row[:, qs], rsb[:], onesrow_32)
                    for ft in range(NPAIR):
                        fs = slice(128 * ft, 128 * ft + 128)
                        ps = ps_mm.tile([128, 512], f32, tag="mm")
                        for ct in range(CT):
                            nc.tensor.matmul(ps[:], wk[:, ct, fs], xpT[:, ct, qs],
                                             start=(ct == 0), stop=False)
                        nc.tensor.matmul(ps[:], nsk[0:1, fs], mukv_row[:, qs],
                                         start=False, stop=not flags["b1"])
                        if flags["b1"]:
                            nc.tensor.matmul(ps[:], wbk[0:1, fs],
                                             stdkv_row[:, qs],
                                             start=False, stop=True)
                        nc.vector.tensor_tensor(out=kT[:, ft, qs], in0=ps[:],
                                                in1=rsb[:], op=OP.mult)
                # column form of 1/std for the V evictions (DRAM bounce)
                scratch_d = dram.tile([T], f32, tag="st_dram")
                nc.sync.dma_start(out=scratch_d[:], in_=rskv_row)
                nc.sync.dma_start(
                    out=rskv_cols[:],
                    in_=scratch_d.rearrange("(j p) -> p j", p=128))

            with tc.tile_pool(name="ph_v", bufs=1) as pv:
                wv = pv.tile([128, CT, C], bf)
                dma_in3(wv, P["wv"].rearrange("(a p) f -> p a f", p=128))
                for tt in range(T // 128):
                    ts_ = slice(128 * tt, 128 * tt + 128)
                    for fc in range(2):
                        fs = slice(512 * fc, 512 * fc + 512)
                        ps = ps_mm.tile([128, 512], f32, tag="mm")
                        for ct in range(CT):
                            nc.tensor.matmul(ps[:], xpT[:, ct, ts_], wv[:, ct, fs],
                                             start=(ct == 0), stop=False)
                        nc.tensor.matmul(ps[:], mukv_row[:, ts_], nsv[0:1, fs],
                                         start=False, stop=not flags["b1"])
                        if flags["b1"]:
                            nc.tensor.matmul(ps[:], stdkv_row[:, ts_],
                                             wbv[0:1, fs], start=False, stop=True)
                        psr = ps[:].rearrange("p (a f) -> p a f", f=128)
                        prs = slice(4 * fc, 4 * fc + 4)
                        if flags["b1"]:
                            wbvb = evw.tile([128, 512], bf, tag="wbvb")
                            bcast_rows(wbv[0:1, fs], wbvb[:], onesrow_bf)
                            wbr = wbvb[:].rearrange("p (a f) -> p a f", f=128)
                            nc.vector.scalar_tensor_tensor(
                                out=v[:, tt, prs, 0:64], in0=psr[:, :, 0:64],
                                scalar=rskv_cols[:, tt:tt + 1],
                                in1=wbr[:, :, 0:64], op0=OP.mult, op1=OP.add)
                            nc.vector.scalar_tensor_tensor(
                                out=v[:, tt, prs, 65:129], in0=psr[:, :, 64:128],
                                scalar=rskv_cols[:, tt:tt + 1],
                                in1=wbr[:, :, 64:128], op0=OP.mult, op1=OP.add)
                        else:
                            nc.vector.tensor_scalar_mul(
                                v[:, tt, prs, 0:64], psr[:, :, 0:64],
                                rskv_cols[:, tt:tt + 1])
                            nc.vector.tensor_scalar_mul(
                                v[:, tt, prs, 65:129], psr[:, :, 64:128],
                                rskv_cols[:, tt:tt + 1])

        # Q matmuls (1/sqrt(D) folded into wq host-side)
        with tc.tile_pool(name="ph_q", bufs=1) as pq, \
             tc.tile_pool(name="evwq", bufs=3) as evwq:
            xTb = pq.tile([128, CT, TOWN], bf)
            dma_in3(xTb, P["xTbf"].rearrange("(a p) t -> p a t", p=128))
            wq = pq.tile([128, CT, C], bf)
            dma_in3(wq, P["wq"].rearrange("(a p) f -> p a f", p=128))
            nsq = pq.tile([1, C], bf)
            nc.sync.dma_start(out=nsq[:], in_=P["nsq"][:, :])
            if flags["b1"]:
                wbq = pq.tile([1, C], bf)
                nc.sync.dma_start(out=wbq[:], in_=P["wbq"][:, :])
            ln_stats(xTb, TOWN, muq_row, rsq_row, None, stdq_row)
            for ci in range(2):
                qs = slice(512 * ci, 512 * ci + 512)
                rsb = statw.tile([128, 512], f32, tag="rsb")
                bcast_rows(rsq_row[:, qs], rsb[:], onesrow_32)
                for ft in range(NPAIR):
                    fs = slice(128 * ft, 128 * ft + 128)
                    ps = ps_mm.tile([128, 512], f32, tag="mm")
                    for ct in range(CT):
                        nc.tensor.matmul(ps[:], wq[:, ct, fs], xTb[:, ct, qs],
                                         start=(ct == 0), stop=False)
                    nc.tensor.matmul(ps[:], nsq[0:1, fs], muq_row[:, qs],
                                     start=False, stop=not flags["b1"])
                    if flags["b1"]:
                        nc.tensor.matmul(ps[:], wbq[0:1, fs], stdq_row[:, qs],
                                         start=False, stop=True)
                    ev = evwq.tile([128, 512], bf, tag="ev")
                    nc.vector.tensor_tensor(out=ev[:], in0=ps[:],
                                            in1=rsb[:], op=OP.mult)
                    nc.sync.dma_start(out=qT_d[:, ft, qs], in_=ev[:])

        # late weights: prefetch during attention (DMA is idle there)
        wproj = wlate.tile([128, NPAIR, C], bf)
        dma_in3(wproj, P["wproj"].rearrange("(a p) f -> p a f", p=128))
        nsf1 = wlate.tile([1, FF], bf)
        nc.sync.dma_start(out=nsf1[:], in_=P["nsf1"][:, :])

        # ------------------------------------------------------------------
        # Phase 2: attention
        # ------------------------------------------------------------------
        with tc.tile_pool(name="amask", bufs=1) as pam, \
             tc.tile_pool(name="aload", bufs=4) as pal, \
             tc.tile_pool(name="awork", bufs=4) as paw, \
             tc.tile_pool(name="anorm", bufs=2) as pad, \
             tc.tile_pool(name="ps_s", bufs=2, space="PSUM") as ps_s, \
             tc.tile_pool(name="ps_y", bufs=1, space="PSUM") as ps_y:
            masksb = pam.tile([128, 16, 512], bf)
            dma_in3(masksb, P["masks"].rearrange("p (a w) -> p a w", w=512))

            def attn_norm_back(pr, ci, psYA, psYB, dde, ddo):
                """Deferred per-pair softmax normalization: PE broadcast of
                the reciprocal denominators + eviction to yT_d.  Emitted
                inside the NEXT pair's slot stream so the in-order PE never
                stalls on the reciprocal chain."""
                qs = slice(512 * ci, 512 * ci + 512)
                # broadcast the reciprocal denominators across partitions on
                # the otherwise-idle GPSIMD engine: no PE matmul, no PSUM
                # bank, no DVE staging copy in the normalization chain
                rbtA = pad.tile([64, 512], f32, tag="rbtA")
                nc.gpsimd.partition_broadcast(rbtA[:], dde[:])
                rbtB = pad.tile([64, 512], f32, tag="rbtB")
                nc.gpsimd.partition_broadcast(rbtB[:], ddo[:])
                yvE = pad.tile([64, 512], bf, tag="yvE")
                nc.vector.tensor_tensor(out=yvE[:], in0=psYA[0:64, :],
                                        in1=rbtA[:], op=OP.mult)
                yvO = pad.tile([64, 512], bf, tag="yvO")
                nc.vector.tensor_tensor(out=yvO[:], in0=psYB[0:64, :],
                                        in1=rbtB[:], op=OP.mult)
                nc.sync.dma_start(out=yT_d[0:64, pr, qs], in_=yvE[:])
                nc.sync.dma_start(out=yT_d[64:128, pr, qs], in_=yvO[:])

            iters = [(pr, ci) for pr in range(NPAIR) for ci in range(2)]
            qts = {}

            def load_qt(k):
                pr_, ci_ = iters[k]
                qt_ = pal.tile([128, 512], bf, tag="qt")
                nc.sync.dma_start(out=qt_[:],
                                  in_=qT_d[:, pr_,
                                           512 * ci_:512 * ci_ + 512])
                qts[k] = qt_

            # keep 3 q loads in flight so no slot ever waits on the DMA
            for k in range(3):
                load_qt(k)

            norm = None
            for it, (pr, ci) in enumerate(iters):
                    qs = slice(512 * ci, 512 * ci + 512)
                    nslot = SLOTS[ci]
                    qt = qts.pop(it)
                    if it + 3 < len(iters):
                        load_qt(it + 3)
                    psYA = ps_y.tile([128, 512], f32, tag="ya")
                    psYB = ps_y.tile([128, 512], f32, tag="yb")

                    def emit_sS_exp(s):
                        js = slice(128 * s, 128 * s + 128)
                        psS = ps_s.tile([128, 1024], f32, tag="s")
                        nc.tensor.matmul(psS[:, 0:512], kT[0:64, pr, js],
                                         qt[0:64, :], start=True, stop=True,
                                         tile_position=(0, 0))
                        nc.tensor.matmul(psS[:, 512:1024],
                                         kT[64:128, pr, js], qt[64:128, :],
                                         start=True, stop=True,
                                         tile_position=(64, 0))
                        pt = paw.tile([128, 1024], bf, tag="p")
                        nc.scalar.activation(pt[:], psS[:], AF.Exp)
                        if ci == 0 or s >= 8:
                            ms = masksb[:, s, :]
                            nc.vector.tensor_tensor(out=pt[:, 0:512],
                                                    in0=pt[:, 0:512], in1=ms,
                                                    op=OP.mult)
                            nc.vector.tensor_tensor(out=pt[:, 512:1024],
                                                    in0=pt[:, 512:1024],
                                                    in1=ms, op=OP.mult)
                        return pt

                    def emit_psY(s, pt):
                        st, sp = (s == 0), (s == nslot - 1)
                        nc.tensor.matmul(psYA[0:65, :], v[:, s, pr, 0:65],
                                         pt[:, 0:512], start=st, stop=sp,
                                         tile_position=(0, 0))
                        nc.tensor.matmul(psYB[0:65, :], v[:, s, pr, 65:130],
                                         pt[:, 512:1024], start=st, stop=sp,
                                         tile_position=(0, 0))

                    # software pipelining: psY(s) is emitted after sS(s+2),
                    # so its exp+mask inputs finished two slots ago and the
                    # in-order PE never waits on the Act/DVE chain.  The
                    # previous pair's deferred normalization lands right
                    # after sS(0), covering slot 0's exp latency.
                    pts = [emit_sS_exp(0)]
                    if norm is not None:
                        norm()
                    pts.append(emit_sS_exp(1))
                    for s in range(2, nslot):
                        pts.append(emit_sS_exp(s))
                        emit_psY(s - 2, pts[s - 2])
                    emit_psY(nslot - 2, pts[nslot - 2])
                    emit_psY(nslot - 1, pts[nslot - 1])

                    dde = pad.tile([1, 512], f32, tag="dde")
                    ddo = pad.tile([1, 512], f32, tag="ddo")
                    de_s = pad.tile([1, 512], f32, tag="de_s")
                    do_s = pad.tile([1, 512], f32, tag="do_s")
                    # custom-DVE ops mishandle base_partition != 0: stage the
                    # partition-64 denominator rows to partition 0 first.
                    nc.vector.tensor_copy(out=de_s[:], in_=psYA[64:65, :])
                    nc.vector.tensor_copy(out=do_s[:], in_=psYB[64:65, :])
                    nc.vector.reciprocal_approx_fast(out=dde[:], in_=de_s[:])
                    nc.vector.reciprocal_approx_fast(out=ddo[:], in_=do_s[:])
                    norm = (lambda pr=pr, ci=ci, psYA=psYA, psYB=psYB,
                            dde=dde, ddo=ddo:
                            attn_norm_back(pr, ci, psYA, psYB, dde, ddo))
            norm()

        if flags.get("dbg"):
            for a in range(NPAIR):
                nc.sync.dma_start(out=P["d_kT"][:, a, :], in_=kT[:, a, :])
            for a in range(T // 128):
                for pr_ in range(NPAIR):
                    nc.sync.dma_start(
                        out=P["d_v"][:, a, 128 * pr_:128 * pr_ + 64],
                        in_=v[:, a, pr_, 0:64])
                    nc.sync.dma_start(
                        out=P["d_v"][:, a, 128 * pr_ + 64:128 * pr_ + 128],
                        in_=v[:, a, pr_, 65:129])
        attn_kv_cm.__exit__(None, None, None)
        # ------------------------------------------------------------------
        # Phase 3: proj + residual, LN2, FF
        # ------------------------------------------------------------------
        xmid = es.enter_context(tc.tile_pool(name="xmid", bufs=1))
        xmT32 = xmid.tile([128, CT, TOWN], f32)
        geluT = xmid.tile([128, FT, TOWN], bf)
        sxm = xmid.tile([128, CT, TOWN], bf)

        pw_cm = tc.tile_pool(name="ph_w1", bufs=1)
        pw = pw_cm.__enter__()
        pp_cm = tc.tile_pool(name="ph_proj", bufs=1)
        pp = pp_cm.__enter__()
        pst_cm = tc.tile_pool(name="pstream", bufs=2)
        pst = pst_cm.__enter__()
        if True:
            # ff weights live in a pool OUTSIDE the proj pool so their
            # plane-DMAs can drain one-per-proj-iteration (the big transfers
            # never block the urgent per-ct y/x loads, and ff1/ff2 start
            # with their weights already resident)
            wff1 = pw.tile([128, CT, FF], bf)
            w1r = P["wff1"].rearrange("(a p) f -> p a f", p=128)
            w2r = P["wff2"].rearrange("(a p) f -> p a f", p=128)
            wdma = [(wff1, w1r, a) for a in range(CT)]
            if flags["gbias"]:
                gb = pw.tile([128, FT], f32)
                nc.sync.dma_start(out=gb[:], in_=P["geluBias"][:, :])
            if flags["bff2"]:
                b2row = pw.tile([1, C], bf)
                nc.sync.dma_start(out=b2row[:], in_=P["bf2row"][:, :])
            if flags["bproj"]:
                bprow = pw.tile([1, C], bf)
                nc.sync.dma_start(out=bprow[:], in_=P["bprow"][:, :])
            if flags["bproj"] or flags["bff2"]:
                onesrow = pw.tile([1, TOWN], bf)
                nc.vector.memset(onesrow, 1.0)
            xmbf = pp.tile([128, CT, TOWN], bf)

            ytfs = []
            for ci in range(2):
                qs = slice(512 * ci, 512 * ci + 512)
                ytf = pst.tile([128, NPAIR, 512], bf, tag="ytf")
                for ft in range(NPAIR):
                    nc.sync.dma_start(out=ytf[:, ft, :], in_=yT_d[:, ft, qs])
                ytfs.append(ytf)

            def proj_chunk(ci):
                qs = slice(512 * ci, 512 * ci + 512)
                ytf = ytfs[ci]
                for ct in range(CT):
                    if wdma:
                        dst, srcr, a = wdma.pop(0)
                        nc.sync.dma_start(out=dst[:, a, :], in_=srcr[:, a, :])
                    cs = slice(128 * ct, 128 * ct + 128)
                    ps = ps_mm.tile([128, 512], f32, tag="mm")
                    for ft in range(NPAIR):
                        nc.tensor.matmul(ps[:], wproj[:, ft, cs],
                                         ytf[:, ft, :], start=(ft == 0),
                                         stop=(ft == NPAIR - 1
                                               and not flags["bproj"]))
                    if flags["bproj"]:
                        nc.tensor.matmul(ps[:], bprow[0:1, cs],
                                         onesrow[0:1, qs],
                                         start=False, stop=True)
                    x32 = pst.tile([128, 512], f32, tag="x32")
                    nc.sync.dma_start(out=x32[:], in_=P["xT32"][cs, qs])
                    nc.vector.tensor_tensor(out=xmT32[:, ct, qs], in0=ps[:],
                                            in1=x32[:], op=OP.add)
                    nc.vector.tensor_copy(out=xmbf[:, ct, qs],
                                          in_=xmT32[:, ct, qs])

            def ln2_rows(ci):
                qs = slice(512 * ci, 512 * ci + 512)
                ln_stats(xmbf, 512, mu2_row_t[0:1, 0:TOWN], rs2_row,
                         None, None, row_off=512 * ci)
                nc.vector.tensor_tensor(out=mursf_row[:, qs],
                                        in0=rs2_row[:, qs],
                                        in1=mu2_row_t[0:1, qs], op=OP.mult)
                nc.vector.tensor_copy(out=murs2_row[:, qs],
                                      in_=mursf_row[:, qs])
                nc.vector.tensor_copy(out=rs2bf_row[:, qs],
                                      in_=rs2_row[:, qs])

            def sxm_chunk(ci):
                qs = slice(512 * ci, 512 * ci + 512)
                rb2 = statw.tile([128, 512], bf, tag="rb2")
                bcast_rows(rs2bf_row[:, qs], rb2[:], onesrow_bf)
                for ct in range(CT):
                    nc.vector.tensor_tensor(out=sxm[:, ct, qs],
                                            in0=xmbf[:, ct, qs], in1=rb2[:],
                                            op=OP.mult)


            def ff1_chunk(ci, wq2):
                qs = slice(512 * ci, 512 * ci + 512)
                for ft in range(FT):
                    fs = slice(128 * ft, 128 * ft + 128)
                    if wq2:
                        dst, srcr, a = wq2.pop(0)
                        nc.sync.dma_start(out=dst[:, a, :], in_=srcr[:, a, :])
                    ps = ps_mm.tile([128, 512], f32, tag="mm")
                    for ct in range(CT):
                        nc.tensor.matmul(ps[:], wff1[:, ct, fs],
                                         sxm[:, ct, qs],
                                         start=(ct == 0), stop=False)
                    nc.tensor.matmul(ps[:], nsf1[0:1, fs], murs2_row[:, qs],
                                     start=False, stop=True)
                    bias = gb[:, ft:ft + 1] if flags["gbias"] else 0.0
                    nc.scalar.activation(geluT[:, ft, qs], ps[:], AF.Gelu,
                                         bias=bias)

            # per-chunk order: chunk 0's LN2 stat chain (Act sqrt + DVE
            # recip) hides under proj(1), chunk 1's under ff1(0); the Act
            # table sequence is sqrt,sqrt,gelu,gelu (1 switch)
            proj_chunk(0)
            ln2_rows(0)
            proj_chunk(1)
            sxm_chunk(0)
            ln2_rows(1)
            ff1_chunk(0, None)
            sxm_chunk(1)

        pst_cm.__exit__(None, None, None)
        pp_cm.__exit__(None, None, None)
        # wff2 lands in the space ph_proj/pstream just freed and streams in
        # under the ff1 matmuls
        with tc.tile_pool(name="ph_w2", bufs=1) as pw2, \
             tc.tile_pool(name="outp", bufs=2) as po:
            wff2 = pw2.tile([128, FT, C], bf)
            wdma2 = [(wff2, w2r, a) for a in range(FT)]

            def ff2_out(ci):
                qs = slice(512 * ci, 512 * ci + 512)
                for ct in range(CT):
                    cs = slice(128 * ct, 128 * ct + 128)
                    ps = ps_mm.tile([128, 512], f32, tag="mm")
                    for ft in range(FT):
                        nc.tensor.matmul(ps[:], wff2[:, ft, cs],
                                         geluT[:, ft, qs],
                                         start=(ft == 0),
                                         stop=(ft == FT - 1
                                               and not flags["bff2"]))
                    if flags["bff2"]:
                        nc.tensor.matmul(ps[:], b2row[0:1, cs],
                                         onesrow[0:1, qs],
                                         start=False, stop=True)
                    ot = po.tile([128, 512], f32, tag="ot")
                    nc.vector.tensor_tensor(out=ot[:], in0=ps[:],
                                            in1=xmT32[:, ct, qs], op=OP.add)
                    nc.sync.dma_start(out=P["outT"][cs, qs], in_=ot[:])

            ff1_chunk(1, wdma2)
            ff2_out(0)
            ff2_out(1)
        pw_cm.__exit__(None, None, None)

        if flags.get("dbg"):
            for a in range(NPAIR):
                nc.sync.dma_start(out=P["d_qT"][:, a, :], in_=qT_d[:, a, :])
                nc.sync.dma_start(out=P["d_yT"][:, a, :], in_=yT_d[:, a, :])
            for a in range(CT):
                nc.sync.dma_start(out=P["d_xm"][128 * a:128 * a + 128, :],
                                  in_=xmT32[:, a, :])
            nc.sync.dma_start(out=P["d_rows"][0:1, 0:T], in_=rskv_row)
            nc.sync.dma_start(out=P["d_rows"][1:2, 0:TOWN], in_=rsq_row)
            nc.sync.dma_start(out=P["d_rows"][2:3, 0:TOWN], in_=rs2_row)
            nc.sync.dma_start(
                out=P["d_rows"][3:4, 0:T].rearrange("o (j p) -> o p j", p=128),
                in_=rskv_cols[:, :])


_WAIT_LIMITS = {
    # walrus codegen encodes sync waits inside the 64B instruction; compute
    # ISA structs only have room for one.  Hoist the overflow onto
    # same-engine NoOps (the sequencer processes waits in program order, so
    # semantics are identical).
    "TensorTensor": 1, "TensorScalarPtr": 1, "Activation": 1, "Matmult": 1,
    "Ldweights": 1, "TensorReduce": 1, "Memset": 1, "TensorCopy": 1,
    "ISA": 1, "Iota": 1, "Reciprocal": 1, "CustomDveAnt": 1, "NoOp": 1,
    "EventSemaphore": 1, "Drain": 1, "DMACopy": 1,
}
_nop_ctr = [0]


def _split_waits(nc):
    import concourse.mybir as mb
    for f in nc.m.functions:
        for bb in f.blocks:
            out = []
            for inst in bb.instructions:
                si = inst.sync_info
                lim = _WAIT_LIMITS.get(getattr(inst, "opcode", None), None)
                if (si is not None and si.on_wait and lim is not None
                        and len(si.on_wait) > lim):
                    waits = list(si.on_wait)
                    extra, keep = waits[:-lim], waits[-lim:]
                    while extra:
                        chunk, extra = extra[:1], extra[1:]
                        _nop_ctr[0] += 1
                        nop = mb.InstEventSemaphore(
                            name=f"I-waitnop-{_nop_ctr[0]}", ins=[], outs=[])
                        nop.engine = inst.engine
                        nop.sync_info = mb.SyncInfo(on_wait=chunk, on_update=[])
                        out.append(nop)
                    inst.sync_info = mb.SyncInfo(on_wait=keep,
                                                 on_update=si.on_update)
                out.append(inst)
            bb.instructions[:] = out


LAST_NC = None
LAST_INMAPS = None


def bench(iters=30):
    """Repeatedly execute the compiled NEFF with device-resident inputs and
    return the min per-iteration wall time in ns (upper bound on HW exec:
    includes PJRT dispatch + axon tunnel overhead, amortized)."""
    import time

    import jax
    import concourse.mybir as mb
    from concourse.bass2jax import (_bass_exec_p, install_neuronx_cc_hook,
                                    Mesh, PartitionSpec, shard_map,
                                    partition_id_tensor)
    from jax.sharding import NamedSharding

    nc, in_maps = LAST_NC, LAST_INMAPS
    assert nc is not None
    install_neuronx_cc_hook()
    pname = nc.partition_id_tensor.name if nc.partition_id_tensor else None
    in_names, out_names, out_avals, zero_outs = [], [], [], []
    for alloc in nc.m.functions[0].allocations:
        if not isinstance(alloc, mb.MemoryLocationSet):
            continue
        name = alloc.memorylocations[0].name
        if alloc.kind == "ExternalInput":
            if name != pname:
                in_names.append(name)
        elif alloc.kind == "ExternalOutput":
            out_names.append(name)
            shape = tuple(alloc.tensor_shape)
            dtype = mb.dt.np(alloc.dtype)
            out_avals.append(jax.core.ShapedArray(shape, dtype))
            zero_outs.append(np.zeros(shape, dtype))
    n_params = len(in_names)
    all_names = in_names + out_names
    if pname is not None:
        all_names = all_names + [pname]

    def _body(*args):
        operands = list(args)
        if pname is not None:
            operands.append(partition_id_tensor())
        return tuple(_bass_exec_p.bind(
            *operands, out_avals=tuple(out_avals), in_names=tuple(all_names),
            out_names=tuple(out_names), lowering_input_output_aliases=(),
            sim_require_finite=True, sim_require_nnan=True, nc=nc))

    devices = jax.devices()[:NCORE]
    mesh = Mesh(np.asarray(devices), ("core",))
    spec = PartitionSpec("core")
    sharded = jax.jit(
        shard_map(_body, mesh=mesh, in_specs=(spec,) * (n_params + len(out_names)),
                  out_specs=(spec,) * len(out_names), check_rep=False),
        keep_unused=True)
    sh = NamedSharding(mesh, spec)
    dev_in = [jax.device_put(
        np.concatenate([np.asarray(in_maps[c][nm]) for c in range(NCORE)], 0), sh)
        for nm in in_names]
    dev_in += [jax.device_put(
        np.concatenate([z] * NCORE, 0), sh) for z in zero_outs]
    out = sharded(*dev_in)
    jax.block_until_ready(out)          # compile + warm
    times = []
    for _ in range(iters):
        t0 = time.perf_counter()
        out = sharded(*dev_in)
        jax.block_until_ready(out)
        times.append(time.perf_counter() - t0)
    times.sort()
    return {"min_ns": int(times[0] * 1e9),
            "p50_ns": int(times[len(times) // 2] * 1e9),
            "times_ms": [round(t * 1e3, 3) for t in times[:5]]}


def bench_amortized(n_lo=8, n_hi=40, reps=8):
    """Per-iteration device time via pipelined dispatch: enqueue n back-to-back
    executions of the compiled NEFF (device-resident inputs), block once at the
    end.  The axon/PJRT dispatch pipeline overlaps RPC latency with device
    execution, so T(n_hi) - T(n_lo) isolates n_hi - n_lo real executions:
    per_iter = (T_hi - T_lo) / (n_hi - n_lo).  Each T is min over `reps`."""
    import time

    import jax
    import concourse.mybir as mb
    from concourse.bass2jax import (_bass_exec_p, install_neuronx_cc_hook,
                                    Mesh, PartitionSpec, shard_map,
                                    partition_id_tensor)
    from jax.sharding import NamedSharding

    nc, in_maps = LAST_NC, LAST_INMAPS
    assert nc is not None
    install_neuronx_cc_hook()
    pname = nc.partition_id_tensor.name if nc.partition_id_tensor else None
    in_names, out_names, out_avals, zero_outs = [], [], [], []
    for alloc in nc.m.functions[0].allocations:
        if not isinstance(alloc, mb.MemoryLocationSet):
            continue
        name = alloc.memorylocations[0].name
        if alloc.kind == "ExternalInput":
            if name != pname:
                in_names.append(name)
        elif alloc.kind == "ExternalOutput":
            out_names.append(name)
            shape = tuple(alloc.tensor_shape)
            dtype = mb.dt.np(alloc.dtype)
            out_avals.append(jax.core.ShapedArray(shape, dtype))
            zero_outs.append(np.zeros(shape, dtype))
    n_params = len(in_names)
    all_names = in_names + out_names
    if pname is not None:
        all_names = all_names + [pname]

    def _body(*args):
        operands = list(args)
        if pname is not None:
            operands.append(partition_id_tensor())
        return tuple(_bass_exec_p.bind(
            *operands, out_avals=tuple(out_avals), in_names=tuple(all_names),
            out_names=tuple(out_names), lowering_input_output_aliases=(),
            sim_require_finite=True, sim_require_nnan=True, nc=nc))

    devices = jax.devices()[:NCORE]
    mesh = Mesh(np.asarray(devices), ("core",))
    spec = PartitionSpec("core")
    sharded = jax.jit(
        shard_map(_body, mesh=mesh, in_specs=(spec,) * (n_params + len(out_names)),
                  out_specs=(spec,) * len(out_names), check_rep=False),
        keep_unused=True)
    sh = NamedSharding(mesh, spec)
    dev_in = [jax.device_put(
        np.concatenate([np.asarray(in_maps[c][nm]) for c in range(NCORE)], 0), sh)
        for nm in in_names]
    dev_in += [jax.device_put(
        np.concatenate([z] * NCORE, 0), sh) for z in zero_outs]
    jax.block_until_ready(sharded(*dev_in))      # compile + warm

    def chain_time(n):
        best = float("inf")
        for _ in range(reps):
            t0 = time.perf_counter()
            outs = [sharded(*dev_in) for _ in range(n)]
            jax.block_until_ready(outs)
            best = min(best, time.perf_counter() - t0)
        return best

    chain_time(n_lo)                              # extra warm for the pipeline
    t_lo = chain_time(n_lo)
    t_hi = chain_time(n_hi)
    per_iter = (t_hi - t_lo) / (n_hi - n_lo)
    return {"per_iter_ns": max(int(per_iter * 1e9), 1),
            "t_lo_ms": round(t_lo * 1e3, 3),
            "t_hi_ms": round(t_hi * 1e3, 3)}


def bench_chain(n_lo=2, n_hi=18, reps=12):
    """Ground-truth device timing: one jitted program executes the NEFF n
    times back-to-back (outT threaded into xT32 to serialize); the timing
    difference between n_hi and n_lo cancels the dispatch/tunnel overhead."""
    import time

    import jax
    import concourse.mybir as mb
    from concourse.bass2jax import (_bass_exec_p, install_neuronx_cc_hook,
                                    Mesh, PartitionSpec, shard_map,
                                    partition_id_tensor)
    from jax.sharding import NamedSharding

    nc, in_maps = LAST_NC, LAST_INMAPS
    assert nc is not None
    install_neuronx_cc_hook()
    pname = nc.partition_id_tensor.name if nc.partition_id_tensor else None
    in_names, out_names, out_avals, zero_outs = [], [], [], []
    for alloc in nc.m.functions[0].allocations:
        if not isinstance(alloc, mb.MemoryLocationSet):
            continue
        name = alloc.memorylocations[0].name
        if alloc.kind == "ExternalInput":
            if name != pname:
                in_names.append(name)
        elif alloc.kind == "ExternalOutput":
            out_names.append(name)
            shape = tuple(alloc.tensor_shape)
            dtype = mb.dt.np(alloc.dtype)
            out_avals.append(jax.core.ShapedArray(shape, dtype))
            zero_outs.append(np.zeros(shape, dtype))
    n_params = len(in_names)
    all_names = in_names + out_names + ([pname] if pname else [])
    x_idx = in_names.index("xT32")
    o_idx = out_names.index("outT")

    def mk_body(n):
        def _body(*args):
            ins = list(args[:n_params])
            zouts = list(args[n_params:])
            for _ in range(n):
                operands = ins + zouts
                if pname is not None:
                    operands.append(partition_id_tensor())
                outs = _bass_exec_p.bind(
                    *operands, out_avals=tuple(out_avals),
                    in_names=tuple(all_names), out_names=tuple(out_names),
                    lowering_input_output_aliases=(),
                    sim_require_finite=True, sim_require_nnan=True, nc=nc)
                ins[x_idx] = outs[o_idx]
            return tuple(outs)
        return _body

    devices = jax.devices()[:NCORE]
    mesh = Mesh(np.asarray(devices), ("core",))
    spec = PartitionSpec("core")
    sh = NamedSharding(mesh, spec)
    dev_in = [jax.device_put(
        np.concatenate([np.asarray(in_maps[c][nm]) for c in range(NCORE)], 0),
        sh) for nm in in_names]
    dev_in += [jax.device_put(np.concatenate([z] * NCORE, 0), sh)
               for z in zero_outs]

    res = {}
    for n in (n_lo, n_hi):
        f = jax.jit(shard_map(mk_body(n), mesh=mesh,
                              in_specs=(spec,) * len(dev_in),
                              out_specs=(spec,) * len(out_names),
                              check_rep=False), keep_unused=True)
        jax.block_until_ready(f(*dev_in))       # compile + warm
        ts = []
        for _ in range(reps):
            t0 = time.perf_counter()
            jax.block_until_ready(f(*dev_in))
            ts.append(time.perf_counter() - t0)
        ts.sort()
        res[n] = ts[0]
    per_iter = (res[n_hi] - res[n_lo]) / (n_hi - n_lo)
    return {"per_iter_ns": int(per_iter * 1e9),
            "t_lo_ms": round(res[n_lo] * 1e3, 2),
            "t_hi_ms": round(res[n_hi] * 1e3, 2)}


def _build_nc(flags):
    nc = bacc.Bacc("TRN2", target_bir_lowering=False, debug=False,
                   num_devices=NCORE)
    P = {}

    def inp(name, shape, d):
        P[name] = nc.declare_dram_parameter(name, list(shape), d, isOutput=False)

    inp("xT32", (C, TOWN), f32)
    inp("xTbf", (C, TOWN), bf)
    inp("xpTbf", (C, T), bf)
    inp("wq", (C, C), bf)
    inp("wk", (C, C), bf)
    inp("wv", (C, C), bf)
    inp("wproj", (C, C), bf)
    inp("wff1", (C, FF), bf)
    inp("wff2", (FF, C), bf)
    inp("nsq", (1, C), bf)
    inp("nsk", (1, C), bf)
    inp("nsv", (1, C), bf)
    inp("nsf1", (1, FF), bf)
    inp("masks", (128, 16 * 512), bf)
    if flags["b1"]:
        inp("wbq", (1, C), bf)
        inp("wbk", (1, C), bf)
        inp("wbv", (1, C), bf)
    if flags["bproj"]:
        inp("bprow", (1, C), bf)
    if flags["gbias"]:
        inp("geluBias", (128, FT), f32)
    if flags["bff2"]:
        inp("bf2row", (1, C), bf)
    P["outT"] = nc.declare_dram_parameter("outT", [C, TOWN], f32, isOutput=True)
    if flags.get("dbg"):
        for nm, shape, d in [("d_kT", [128, NPAIR, T], bf),
                             ("d_qT", [128, NPAIR, TOWN], bf),
                             ("d_v", [128, T // 128, C], bf),
                             ("d_yT", [128, NPAIR, TOWN], bf),
                             ("d_xm", [C, TOWN], f32),
                             ("d_rows", [8, T], f32),
                             ("d_S", [8, 128, 1024], f32),
                             ("d_P", [8, 128, 1024], bf),
                             ("d_ypre", [128, 1024], f32)]:
            P[nm] = nc.declare_dram_parameter(nm, shape, d, isOutput=True)

    with tile.TileContext(nc, pool_alloc_mode="queue") as tc:
        _emit(tc, P, flags)
    nc.compile()
    return nc


# --------------------------------------------------------------------------
# host side
# --------------------------------------------------------------------------

def _own_rows(half):
    a, b = BLOCKS[half]
    return np.concatenate([np.arange(512 * a, 512 * a + 512),
                           np.arange(512 * b, 512 * b + 512)])


def _mask_pack(half):
    """[128, 16*512] bf16; col-block s = keep-mask for key-tile slot s."""
    out = np.ones((128, 16 * 512), dtype=F32)
    jj = np.arange(128)[:, None]
    qq = np.arange(512)[None, :]
    a, b = BLOCKS[half]
    for s in range(8):
        out[:, 512 * s:512 * s + 512] = (128 * s + jj) <= (512 * a + qq)
    for s in range(8, 16):
        out[:, 512 * s:512 * s + 512] = (128 * s + jj) <= (512 * b + qq)
    return out.astype(BF16)


def kernel(**inputs):
    global LAST_RESULT
    ins = {k: np.asarray(v) for k, v in inputs.items()}
    x = ins["x"].astype(F32)
    perm = np.asarray(ins["perm"]).astype(np.int64)
    Wqkv, Wproj = ins["Wqkv"].astype(F32), ins["Wproj"].astype(F32)
    bproj = ins["bproj"].astype(F32)
    g1, b1 = ins["ln1_g"].astype(F32), ins["ln1_b"].astype(F32)
    g2, b2 = ins["ln2_g"].astype(F32), ins["ln2_b"].astype(F32)
    Wff1, bff1 = ins["Wff1"].astype(F32), ins["bff1"].astype(F32)
    Wff2, bff2 = ins["Wff2"].astype(F32), ins["bff2"].astype(F32)

    sigma = np.argsort(perm)
    sc = 1.0 / np.sqrt(D)

    wq_f = Wqkv[:, :C] * g1[:, None] * sc
    wk_f = Wqkv[:, C:2 * C] * g1[:, None]
    wv_f = Wqkv[:, 2 * C:] * g1[:, None]
    wf1_f = Wff1 * g2[:, None]

    flags = {
        "b1": bool(np.any(b1 != 0.0)),
        "bproj": bool(np.any(bproj != 0.0)),
        "gbias": bool(np.any(bff1 != 0.0) or np.any(b2 != 0.0)),
        "bff2": bool(np.any(bff2 != 0.0)),
        "dbg": bool(os.environ.get("KDBG")),
    }

    shared = {
        "wq": wq_f.astype(BF16), "wk": wk_f.astype(BF16),
        "wv": wv_f.astype(BF16), "wproj": Wproj.astype(BF16),
        "wff1": wf1_f.astype(BF16), "wff2": Wff2.astype(BF16),
        "nsq": (-wq_f.sum(0))[None, :].astype(BF16),
        "nsk": (-wk_f.sum(0))[None, :].astype(BF16),
        "nsv": (-wv_f.sum(0))[None, :].astype(BF16),
        "nsf1": (-wf1_f.sum(0))[None, :].astype(BF16),
    }
    if flags["b1"]:
        shared["wbq"] = (b1 @ Wqkv[:, :C] * sc)[None, :].astype(BF16)
        shared["wbk"] = (b1 @ Wqkv[:, C:2 * C])[None, :].astype(BF16)
        shared["wbv"] = (b1 @ Wqkv[:, 2 * C:])[None, :].astype(BF16)
    if flags["bproj"]:
        shared["bprow"] = bproj[None, :].astype(BF16)
    if flags["gbias"]:
        gb = (bff1 + b2 @ Wff1).astype(F32)           # [FF]
        shared["geluBias"] = np.ascontiguousarray(
            gb.reshape(FT, 128).T).astype(F32)        # [128, FT]
    if flags["bff2"]:
        shared["bf2row"] = bff2[None, :].astype(BF16)

    in_maps = []
    for c in range(NCORE):
        b, half = c // 2, c % 2
        rows_ = _own_rows(half)
        xb = x[b]
        xq = xb[rows_]
        m = dict(shared)
        m["xT32"] = np.ascontiguousarray(xq.T)
        m["xTbf"] = m["xT32"].astype(BF16)
        m["xpTbf"] = np.ascontiguousarray(xb[sigma].T).astype(BF16)
        m["masks"] = _mask_pack(half)
        in_maps.append(m)

    global LAST_NC, LAST_INMAPS
    nc = _build_nc(flags)
    LAST_NC, LAST_INMAPS = nc, in_maps
    res = run_bass_kernel_spmd(nc, in_maps, list(range(NCORE)))
    LAST_RESULT = res

    out = np.empty((B, T, C), dtype=F32)
    for c in range(NCORE):
        b, half = c // 2, c % 2
        out[b, _own_rows(half)] = res.results[c]["outT"].T
    return out


if __name__ == "__main__":
    rng = np.random.default_rng(0)
    demo = {
        "x": rng.standard_normal((B, T, C), dtype=F32),
        "perm": rng.permutation(T).astype(np.int32),
        "Wqkv": rng.standard_normal((C, 3 * C), dtype=F32) / 32,
        "Wproj": rng.standard_normal((C, C), dtype=F32) / 32,
        "bproj": np.zeros(C, F32),
        "ln1_g": np.ones(C, F32), "ln1_b": np.zeros(C, F32),
        "ln2_g": np.ones(C, F32), "ln2_b": np.zeros(C, F32),
        "Wff1": rng.standard_normal((C, FF), dtype=F32) / 32,
        "bff1": np.zeros(FF, F32),
        "Wff2": rng.standard_normal((FF, C), dtype=F32) / 45,
        "bff2": np.zeros(C, F32),
    }
    o = kernel(**demo)
    print("ok", o.shape, o.dtype)

